# revision 4
# baseline (speedup 1.0000x reference)
"""Multi-head attention (B=4, S=2048, D=1024, H=16, causal) on 8 trn2 cores.

Sharding: data-parallel over batch (4) x tensor-parallel over head groups (2).
Core c handles batch b=c//2, heads g=c%2 (8 heads each). Each core computes
its partial output projection; host sums the two partials per batch and adds
the bias.

Per-core pipeline (all matmul inputs fp16, fp32 accumulation):
  1. qT/kT = W.T @ X.T   [512, 2048] (head-major rows), v = X @ Wv [2048, 512]
  2. per (head, 128-row s-block): scores[s,t] = qT.T kT (K=64 matmul),
     causal mask-add on PSUM (DVE), row-max (DVE), exp(bias=-max) with
     accum_out denominator (ACT), normalize (GPSIMD) -> P fp16
  3. P -> P^T via DMA-transpose XBAR (fp16), PV: out^T[dk, s] += v_tile.T P^T
  4. y_partial[s, :] = concat^T.T @ Wo_part  (K=512), fp32 psum -> DRAM

The tensor-engine MM/LDW ISA slots hold a single semaphore wait, so every
matmul's dependencies are funneled through one engine: DMA-loaded tiles get a
full-range in-place DVE "touch" after load, and the P^T tile gets a GPSIMD
touch after the transposes, so each MM waits on at most one proc.
"""

import math

import numpy as np

B, S, D, H = 4, 2048, 1024, 16
DK = 64
HLOC = 8          # heads per core
HD = HLOC * DK    # 512 local concat dims
P = 128
SBLKS = S // P    # 16
CH = 512          # score/psum chunk width
SCHUNKS = S // CH  # 4
KO = D // P       # 8 contraction tiles for projections
MPAIRS = 4        # head pairs per core (qT/kT stored as [128, MPAIRS, S])
NEG = -30000.0


def build():
    import concourse.bass as bass
    import concourse.mybir as mybir
    import concourse.tile as tile
    from concourse import bacc

    fp16 = mybir.dt.float16
    f32 = mybir.dt.float32

    nc = bacc.Bacc()

    xtq = nc.dram_tensor("xtq", [D, S], fp16, kind="ExternalInput")
    xtk = nc.dram_tensor("xtk", [D, S], fp16, kind="ExternalInput")
    xtv = nc.dram_tensor("xtv", [D, S], fp16, kind="ExternalInput")
    wq = nc.dram_tensor("wq", [D, HD], fp16, kind="ExternalInput")
    wk = nc.dram_tensor("wk", [D, HD], fp16, kind="ExternalInput")
    wv = nc.dram_tensor("wv", [D, HD], fp16, kind="ExternalInput")
    wo = nc.dram_tensor("wo", [HD, D], fp16, kind="ExternalInput")
    maskadd = nc.dram_tensor("maskadd", [P, 4, CH], f32, kind="ExternalInput")
    y = nc.dram_tensor("y", [S, D], f32, kind="ExternalOutput")

    with tile.TileContext(nc) as tc:
        with (
            tc.tile_pool(name="persist", bufs=1) as persist,
            tc.tile_pool(name="pssc", bufs=6, space="PSUM") as pssc,
            tc.tile_pool(name="psmm", bufs=2, space="PSUM") as psmm,
            tc.tile_pool(name="stats", bufs=24) as stats,
        ):
            # ---- constants / weights ----
            mask_sb = persist.tile([P, 4, CH], f32, tag="mask")
            nc.sync.dma_start(out=mask_sb, in_=maskadd[:])

            wq_sb = persist.tile([P, KO, HD], fp16, tag="wq")
            wk_sb = persist.tile([P, KO, HD], fp16, tag="wk")
            wv_sb = persist.tile([P, KO, HD], fp16, tag="wv")
            nc.sync.dma_start(out=wq_sb, in_=wq[:].rearrange("(ko p) n -> p ko n", p=P))
            nc.sync.dma_start(out=wk_sb, in_=wk[:].rearrange("(ko p) n -> p ko n", p=P))
            nc.sync.dma_start(out=wv_sb, in_=wv[:].rearrange("(ko p) n -> p ko n", p=P))
            wo_sb = persist.tile([P, MPAIRS, D], fp16, tag="wo")
            nc.sync.dma_start(out=wo_sb, in_=wo[:].rearrange("(m p) n -> p m n", p=P))

            # ---- persistent activations ----
            qt = persist.tile([P, MPAIRS, S], fp16, tag="qt")   # rows = hd % 128
            kt = persist.tile([P, MPAIRS, S], fp16, tag="kt")
            vv = persist.tile([P, SBLKS, HD], fp16, tag="vv")   # [t%128, t//128, hd]
            outt = persist.tile([P, MPAIRS, S], fp16, tag="outt")  # concat^T

            # ---- phase 1: projections ----
            with tc.tile_pool(name="xt", bufs=2) as xtpool:
                for name, src, wsb, dstq in (
                    ("q", xtq, wq_sb, qt),
                    ("k", xtk, wk_sb, kt),
                ):
                    xsb = xtpool.tile([P, KO, S], fp16, tag="xt")
                    src_r = src[:].rearrange("(ko p) s -> p ko s", p=P)
                    for ko in range(KO):
                        nc.sync.dma_start(
                            out=xsb[:, ko, :], in_=src_r[:, ko, :]
                        )
                    for m in range(MPAIRS):
                        for nch in range(SCHUNKS):
                            ps = psmm.tile([P, CH], f32, tag="ps")
                            for ko in range(KO):
                                nc.tensor.matmul(
                                    ps,
                                    lhsT=wsb[:, ko, m * P : (m + 1) * P],
                                    rhs=xsb[:, ko, nch * CH : (nch + 1) * CH],
                                    start=(ko == 0),
                                    stop=(ko == KO - 1),
                                )
                            nc.vector.tensor_copy(
                                out=dstq[:, m, nch * CH : (nch + 1) * CH], in_=ps
                            )
                # v projection: lhsT = X^T tile, rhs = Wv -> v[t, hd]
                xsb = xtpool.tile([P, KO, S], fp16, tag="xt")
                src_r = xtv[:].rearrange("(ko p) s -> p ko s", p=P)
                for ko in range(KO):
                    nc.sync.dma_start(
                        out=xsb[:, ko, :], in_=src_r[:, ko, :]
                    )
                for tm in range(SBLKS):
                    ps = psmm.tile([P, CH], f32, tag="ps")
                    for ko in range(KO):
                        nc.tensor.matmul(
                            ps,
                            lhsT=xsb[:, ko, tm * P : (tm + 1) * P],
                            rhs=wv_sb[:, ko, :],
                            start=(ko == 0),
                            stop=(ko == KO - 1),
                        )
                    nc.vector.tensor_copy(out=vv[:, tm, :], in_=ps)

            # ---- phase 2: attention, two heads of a pair interleaved ----
            ctx2 = tc.tile_pool(name="escr", bufs=3)
            escr = ctx2.__enter__()
            ctx3 = tc.tile_pool(name="pt", bufs=2)
            ptpool = ctx3.__enter__()
            ctx4 = tc.tile_pool(name="outbuf", bufs=2)
            outbuf = ctx4.__enter__()
            for c in range(SCHUNKS):
                for m in range(MPAIRS):
                    nblk = 4 * (c + 1)
                    pts = [
                        ptpool.tile([P, SBLKS, CH], fp16, tag=f"pt{z}",
                                    name=f"pt{z}")
                        for z in (0, 1)
                    ]
                    for r in range(4):
                        i = 4 * c + r  # s-block index
                        wl = P * (r + 1)  # causal width of the diagonal chunk
                        width = c * CH + wl
                        for z in (0, 1):
                            off = z * 64
                            scz = []
                            for cc in range(c + 1):
                                w = CH if cc < c else wl
                                ps = pssc.tile([P, CH], f32, tag="ps")
                                nc.tensor.matmul(
                                    ps[:, :w],
                                    lhsT=qt[off : off + 64, m, i * P : (i + 1) * P],
                                    rhs=kt[off : off + 64, m, cc * CH : cc * CH + w],
                                    start=True,
                                    stop=True,
                                )
                                scz.append(ps)
                            # causal mask on the diagonal chunk
                            nc.vector.tensor_tensor(
                                out=scz[c][:, :wl], in0=scz[c][:, :wl],
                                in1=mask_sb[:, r, :wl], op=mybir.AluOpType.add,
                            )
                            # negated row max (exp bias)
                            negmx = stats.tile([P, 1], f32, tag="negmx")
                            if c == 0:
                                nc.vector.reduce_max(
                                    negmx, scz[0][:, :wl],
                                    axis=mybir.AxisListType.X, negate=True,
                                )
                            else:
                                mxarr = stats.tile([P, 4], f32, tag="mxarr")
                                for cc in range(c + 1):
                                    w = CH if cc < c else wl
                                    nc.vector.reduce_max(
                                        mxarr[:, cc : cc + 1], scz[cc][:, :w],
                                        axis=mybir.AxisListType.X,
                                    )
                                nc.vector.reduce_max(
                                    negmx, mxarr[:, 0 : c + 1],
                                    axis=mybir.AxisListType.X, negate=True,
                                )
                            # exp + accumulate denominator
                            ebuf = escr.tile(
                                [P, SCHUNKS * CH], fp16, tag=f"ebuf{z}",
                                name=f"ebuf{z}",
                            )
                            acc = stats.tile([P, 4], f32, tag="acc")
                            for cc in range(c + 1):
                                w = CH if cc < c else wl
                                nc.scalar.activation(
                                    out=ebuf[:, cc * CH : cc * CH + w],
                                    in_=scz[cc][:, :w],
                                    func=mybir.ActivationFunctionType.Exp,
                                    bias=negmx,
                                    scale=1.0,
                                    accum_out=acc[:, cc : cc + 1],
                                )
                            den = stats.tile([P, 1], f32, tag="den")
                            if c == 0:
                                nc.vector.reciprocal(den, acc[:, 0:1])
                            else:
                                nc.vector.reduce_sum(
                                    den, acc[:, 0 : c + 1],
                                    axis=mybir.AxisListType.X,
                                )
                                nc.vector.reciprocal(den, den)
                            # normalize P = E/den; split across DVE / gpsimd
                            nc.gpsimd.tensor_scalar_mul(
                                ebuf[:, 0:width], ebuf[:, 0:width], den
                            )
                            # transpose P[s-block, t] -> P^T[t, s-block cols]
                            nc.sync.dma_start(
                                out=pts[z][:, 0 : i + 1, r * P : (r + 1) * P],
                                in_=ebuf[:, 0:width],
                                transpose=True,
                            )
                            # zero t-blocks above the causal limit
                            if i + 1 < nblk:
                                nc.gpsimd.memset(
                                    pts[z][:, i + 1 : nblk, r * P : (r + 1) * P],
                                    0.0,
                                )
                    # PV for this s-chunk: out^T[dk, s] = sum_j v_j.T @ P^T_j
                    for z in (0, 1):
                        off = z * 64
                        po = psmm.tile([64, CH], f32, tag="ps")
                        for half in (0, 1):
                            hs = half * 256
                            for j in range(nblk):
                                nc.tensor.matmul(
                                    po[:, hs : hs + 256],
                                    lhsT=vv[:, j, off + m * P : off + m * P + 64],
                                    rhs=pts[z][:, j, hs : hs + 256],
                                    start=(j == 0),
                                    stop=(j == nblk - 1),
                                )
                        nc.scalar.copy(
                            out=outt[off : off + 64, m, c * CH : (c + 1) * CH],
                            in_=po,
                        )

                # ---- output projection for this chunk's 4 s-blocks ----
                for i in range(4 * c, 4 * c + 4):
                    ysb = outbuf.tile([P, D], f32, tag="ysb", name="ysb")
                    for nch in range(2):
                        ps = psmm.tile([P, CH], f32, tag="ps", name="ps")
                        for m in range(MPAIRS):
                            nc.tensor.matmul(
                                ps,
                                lhsT=outt[:, m, i * P : (i + 1) * P],
                                rhs=wo_sb[:, m, nch * CH : (nch + 1) * CH],
                                start=(m == 0),
                                stop=(m == MPAIRS - 1),
                            )
                        nc.scalar.copy(
                            out=ysb[:, nch * CH : (nch + 1) * CH], in_=ps
                        )
                    nc.sync.dma_start(out=y[:][i * P : (i + 1) * P, :], in_=ysb)
            ctx4.__exit__(None, None, None)
            ctx3.__exit__(None, None, None)
            ctx2.__exit__(None, None, None)

    nc.finalize()
    return nc


def _prep_inputs(Q, K, V, Wq, Wk, Wv, Wo):
    """Host-side shard + layout prep. Returns list of 8 in_maps."""
    rt8 = math.sqrt(math.sqrt(64.0))  # sqrt(8): scale split over q and k
    in_maps = []
    mask = np.zeros((P, 4, CH), np.float32)
    for r in range(4):
        x = np.arange(P)[:, None]
        yy = np.arange(CH)[None, :]
        mask[:, r, :] = np.where(x - yy + 128 * r >= 0, 0.0, NEG)
    for c in range(8):
        b, g = c // 2, c % 2
        heads = slice(g * HLOC, (g + 1) * HLOC)
        # [H,D,DK] -> [D, HLOC*DK]
        wq_p = (Wq[heads] * rt8).transpose(1, 0, 2).reshape(D, HD)
        wk_p = (Wk[heads] * rt8).transpose(1, 0, 2).reshape(D, HD)
        wv_p = Wv[heads].transpose(1, 0, 2).reshape(D, HD)
        wo_p = Wo[:, g * HD : (g + 1) * HD].T  # [HD, D]
        in_maps.append({
            "xtq": np.ascontiguousarray(Q[b].T).astype(np.float16),
            "xtk": np.ascontiguousarray(K[b].T).astype(np.float16),
            "xtv": np.ascontiguousarray(V[b].T).astype(np.float16),
            "wq": np.ascontiguousarray(wq_p).astype(np.float16),
            "wk": np.ascontiguousarray(wk_p).astype(np.float16),
            "wv": np.ascontiguousarray(wv_p).astype(np.float16),
            "wo": np.ascontiguousarray(wo_p).astype(np.float16),
            "maskadd": mask,
        })
    return in_maps


_NC = []


def kernel(Q, K, V, mask, Wq, Wk, Wv, Wo, bo, _trace=False):
    from concourse.bass_utils import run_bass_kernel_spmd

    Q, K, V = np.asarray(Q), np.asarray(K), np.asarray(V)
    Wq, Wk, Wv = np.asarray(Wq), np.asarray(Wk), np.asarray(Wv)
    Wo, bo = np.asarray(Wo), np.asarray(bo)

    if not _NC:
        _NC.append(build())
    nc = _NC[0]
    in_maps = _prep_inputs(Q, K, V, Wq, Wk, Wv, Wo)
    res = run_bass_kernel_spmd(nc, in_maps, core_ids=list(range(8)), trace=_trace)
    ys = [r["y"] for r in res.results]
    out = np.stack([ys[2 * b] + ys[2 * b + 1] for b in range(B)])
    out = out + bo[None, None, :].astype(np.float32)
    if _trace:
        kernel._last = res
    return out.astype(np.float32)



# revision 5
# speedup vs baseline: 1.0436x; 1.0436x over previous
"""Multi-head attention (B=4, S=2048, D=1024, H=16, causal) on 8 trn2 cores.

Sharding: data-parallel over batch (4) x tensor-parallel over head groups (2).
Core c handles batch b=c//2, heads g=c%2 (8 heads each). Each core computes
its partial output projection; host sums the two partials per batch and adds
the bias.

Per-core pipeline (all matmul inputs fp16, fp32 accumulation):
  1. qT/kT = W.T @ X.T   [512, 2048] (head-major rows), v = X @ Wv [2048, 512]
  2. per (head, 128-row s-block): scores[s,t] = qT.T kT (K=64 matmul),
     causal mask-add on PSUM (DVE), row-max (DVE), exp(bias=-max) with
     accum_out denominator (ACT), normalize (GPSIMD) -> P fp16
  3. P -> P^T via DMA-transpose XBAR (fp16), PV: out^T[dk, s] += v_tile.T P^T
  4. y_partial[s, :] = concat^T.T @ Wo_part  (K=512), fp32 psum -> DRAM

The tensor-engine MM/LDW ISA slots hold a single semaphore wait, so every
matmul's dependencies are funneled through one engine: DMA-loaded tiles get a
full-range in-place DVE "touch" after load, and the P^T tile gets a GPSIMD
touch after the transposes, so each MM waits on at most one proc.
"""

import math

import numpy as np

B, S, D, H = 4, 2048, 1024, 16
DK = 64
HLOC = 8          # heads per core
HD = HLOC * DK    # 512 local concat dims
P = 128
SBLKS = S // P    # 16
CH = 512          # score/psum chunk width
SCHUNKS = S // CH  # 4
KO = D // P       # 8 contraction tiles for projections
MPAIRS = 4        # head pairs per core (qT/kT stored as [128, MPAIRS, S])
NEG = -30000.0


def build():
    import concourse.bass as bass
    import concourse.mybir as mybir
    import concourse.tile as tile
    from concourse import bacc

    fp16 = mybir.dt.float16
    f32 = mybir.dt.float32

    nc = bacc.Bacc()

    xtq = nc.dram_tensor("xtq", [D, S], fp16, kind="ExternalInput")
    xtk = nc.dram_tensor("xtk", [D, S], fp16, kind="ExternalInput")
    xtv = nc.dram_tensor("xtv", [D, S], fp16, kind="ExternalInput")
    wq = nc.dram_tensor("wq", [D, HD], fp16, kind="ExternalInput")
    wk = nc.dram_tensor("wk", [D, HD], fp16, kind="ExternalInput")
    wv = nc.dram_tensor("wv", [D, HD], fp16, kind="ExternalInput")
    wo = nc.dram_tensor("wo", [HD, D], fp16, kind="ExternalInput")
    maskadd = nc.dram_tensor("maskadd", [P, 4, CH], f32, kind="ExternalInput")
    y = nc.dram_tensor("y", [S, D], f32, kind="ExternalOutput")

    with tile.TileContext(nc) as tc:
        with (
            tc.tile_pool(name="persist", bufs=1) as persist,
            tc.tile_pool(name="pssc", bufs=6, space="PSUM") as pssc,
            tc.tile_pool(name="psmm", bufs=2, space="PSUM") as psmm,
            tc.tile_pool(name="stats", bufs=24) as stats,
        ):
            # ---- constants / weights ----
            mask_sb = persist.tile([P, 4, CH], f32, tag="mask")
            nc.sync.dma_start(out=mask_sb, in_=maskadd[:])

            wq_sb = persist.tile([P, KO, HD], fp16, tag="wq")
            wk_sb = persist.tile([P, KO, HD], fp16, tag="wk")
            wv_sb = persist.tile([P, KO, HD], fp16, tag="wv")
            nc.sync.dma_start(out=wq_sb, in_=wq[:].rearrange("(ko p) n -> p ko n", p=P))
            nc.sync.dma_start(out=wk_sb, in_=wk[:].rearrange("(ko p) n -> p ko n", p=P))
            nc.sync.dma_start(out=wv_sb, in_=wv[:].rearrange("(ko p) n -> p ko n", p=P))
            wo_sb = persist.tile([P, MPAIRS, D], fp16, tag="wo")
            nc.sync.dma_start(out=wo_sb, in_=wo[:].rearrange("(m p) n -> p m n", p=P))

            # ---- persistent activations ----
            qt = persist.tile([P, MPAIRS, S], fp16, tag="qt")   # rows = hd % 128
            kt = persist.tile([P, MPAIRS, S], fp16, tag="kt")
            vv = persist.tile([P, SBLKS, HD], fp16, tag="vv")   # [t%128, t//128, hd]
            outt = persist.tile([P, MPAIRS, S], fp16, tag="outt")  # concat^T

            # ---- phase 1: projections ----
            with tc.tile_pool(name="xt", bufs=2) as xtpool:
                for name, src, wsb, dstq in (
                    ("q", xtq, wq_sb, qt),
                    ("k", xtk, wk_sb, kt),
                ):
                    xsb = xtpool.tile([P, KO, S], fp16, tag="xt")
                    src_r = src[:].rearrange("(ko p) s -> p ko s", p=P)
                    for ko in range(KO):
                        nc.sync.dma_start(
                            out=xsb[:, ko, :], in_=src_r[:, ko, :]
                        )
                    for m in range(MPAIRS):
                        for nch in range(SCHUNKS):
                            ps = psmm.tile([P, CH], f32, tag="ps")
                            for ko in range(KO):
                                nc.tensor.matmul(
                                    ps,
                                    lhsT=wsb[:, ko, m * P : (m + 1) * P],
                                    rhs=xsb[:, ko, nch * CH : (nch + 1) * CH],
                                    start=(ko == 0),
                                    stop=(ko == KO - 1),
                                )
                            nc.vector.tensor_copy(
                                out=dstq[:, m, nch * CH : (nch + 1) * CH], in_=ps
                            )
                # v projection: lhsT = X^T tile, rhs = Wv -> v[t, hd]
                xsb = xtpool.tile([P, KO, S], fp16, tag="xt")
                src_r = xtv[:].rearrange("(ko p) s -> p ko s", p=P)
                for ko in range(KO):
                    nc.sync.dma_start(
                        out=xsb[:, ko, :], in_=src_r[:, ko, :]
                    )
                for tm in range(SBLKS):
                    ps = psmm.tile([P, CH], f32, tag="ps")
                    for ko in range(KO):
                        nc.tensor.matmul(
                            ps,
                            lhsT=xsb[:, ko, tm * P : (tm + 1) * P],
                            rhs=wv_sb[:, ko, :],
                            start=(ko == 0),
                            stop=(ko == KO - 1),
                        )
                    nc.vector.tensor_copy(out=vv[:, tm, :], in_=ps)

            # ---- phase 2: attention, two heads of a pair interleaved ----
            ctx2 = tc.tile_pool(name="escr", bufs=3)
            escr = ctx2.__enter__()
            ctx3 = tc.tile_pool(name="pt", bufs=2)
            ptpool = ctx3.__enter__()
            ctx4 = tc.tile_pool(name="outbuf", bufs=2)
            outbuf = ctx4.__enter__()
            for c in range(SCHUNKS):
                for m in range(MPAIRS):
                    nblk = 4 * (c + 1)
                    pts = [
                        ptpool.tile([P, SBLKS, CH], fp16, tag=f"pt{z}",
                                    name=f"pt{z}")
                        for z in (0, 1)
                    ]
                    for r in range(4):
                        i = 4 * c + r  # s-block index
                        wl = P * (r + 1)  # causal width of the diagonal chunk
                        width = c * CH + wl
                        for z in (0, 1):
                            off = z * 64
                            scz = []
                            for cc in range(c + 1):
                                w = CH if cc < c else wl
                                ps = pssc.tile([P, CH], f32, tag="ps")
                                nc.tensor.matmul(
                                    ps[:, :w],
                                    lhsT=qt[off : off + 64, m, i * P : (i + 1) * P],
                                    rhs=kt[off : off + 64, m, cc * CH : cc * CH + w],
                                    start=True,
                                    stop=True,
                                )
                                scz.append(ps)
                            # causal mask on the diagonal 128-col block
                            nc.vector.tensor_tensor(
                                out=scz[c][:, wl - P : wl],
                                in0=scz[c][:, wl - P : wl],
                                in1=mask_sb[:, 0, :P], op=mybir.AluOpType.add,
                            )
                            # negated row max (exp bias)
                            negmx = stats.tile([P, 1], f32, tag="negmx")
                            if c == 0:
                                nc.vector.reduce_max(
                                    negmx, scz[0][:, :wl],
                                    axis=mybir.AxisListType.X, negate=True,
                                )
                            else:
                                mxarr = stats.tile([P, 4], f32, tag="mxarr")
                                for cc in range(c + 1):
                                    w = CH if cc < c else wl
                                    nc.vector.reduce_max(
                                        mxarr[:, cc : cc + 1], scz[cc][:, :w],
                                        axis=mybir.AxisListType.X,
                                    )
                                nc.vector.reduce_max(
                                    negmx, mxarr[:, 0 : c + 1],
                                    axis=mybir.AxisListType.X, negate=True,
                                )
                            # exp + accumulate denominator
                            ebuf = escr.tile(
                                [P, SCHUNKS * CH], fp16, tag=f"ebuf{z}",
                                name=f"ebuf{z}",
                            )
                            acc = stats.tile([P, 4], f32, tag="acc")
                            for cc in range(c + 1):
                                w = CH if cc < c else wl
                                nc.scalar.activation(
                                    out=ebuf[:, cc * CH : cc * CH + w],
                                    in_=scz[cc][:, :w],
                                    func=mybir.ActivationFunctionType.Exp,
                                    bias=negmx,
                                    scale=1.0,
                                    accum_out=acc[:, cc : cc + 1],
                                )
                            den = stats.tile([P, 1], f32, tag="den")
                            if c == 0:
                                nc.vector.reciprocal(den, acc[:, 0:1])
                            else:
                                nc.vector.reduce_sum(
                                    den, acc[:, 0 : c + 1],
                                    axis=mybir.AxisListType.X,
                                )
                                nc.vector.reciprocal(den, den)
                            # normalize P = E/den; split across DVE / gpsimd
                            nc.gpsimd.tensor_scalar_mul(
                                ebuf[:, 0:width], ebuf[:, 0:width], den
                            )
                            # transpose P[s-block, t] -> P^T[t, s-block cols]
                            nc.sync.dma_start(
                                out=pts[z][:, 0 : i + 1, r * P : (r + 1) * P],
                                in_=ebuf[:, 0:width],
                                transpose=True,
                            )
                            # zero t-blocks above the causal limit
                            if i + 1 < nblk:
                                nc.gpsimd.memset(
                                    pts[z][:, i + 1 : nblk, r * P : (r + 1) * P],
                                    0.0,
                                )
                    # PV for this s-chunk: out^T[dk, s] = sum_j v_j.T @ P^T_j
                    for z in (0, 1):
                        off = z * 64
                        po = psmm.tile([64, CH], f32, tag="ps")
                        for half in (0, 1):
                            hs = half * 256
                            for j in range(nblk):
                                nc.tensor.matmul(
                                    po[:, hs : hs + 256],
                                    lhsT=vv[:, j, off + m * P : off + m * P + 64],
                                    rhs=pts[z][:, j, hs : hs + 256],
                                    start=(j == 0),
                                    stop=(j == nblk - 1),
                                )
                        nc.scalar.copy(
                            out=outt[off : off + 64, m, c * CH : (c + 1) * CH],
                            in_=po,
                        )

                # ---- output projection for this chunk's 4 s-blocks ----
                for i in range(4 * c, 4 * c + 4):
                    ysb = outbuf.tile([P, D], f32, tag="ysb", name="ysb")
                    for nch in range(2):
                        ps = psmm.tile([P, CH], f32, tag="ps", name="ps")
                        for m in range(MPAIRS):
                            nc.tensor.matmul(
                                ps,
                                lhsT=outt[:, m, i * P : (i + 1) * P],
                                rhs=wo_sb[:, m, nch * CH : (nch + 1) * CH],
                                start=(m == 0),
                                stop=(m == MPAIRS - 1),
                            )
                        nc.scalar.copy(
                            out=ysb[:, nch * CH : (nch + 1) * CH], in_=ps
                        )
                    nc.sync.dma_start(out=y[:][i * P : (i + 1) * P, :], in_=ysb)
            ctx4.__exit__(None, None, None)
            ctx3.__exit__(None, None, None)
            ctx2.__exit__(None, None, None)

    nc.finalize()
    return nc


def _prep_inputs(Q, K, V, Wq, Wk, Wv, Wo):
    """Host-side shard + layout prep. Returns list of 8 in_maps."""
    rt8 = math.sqrt(math.sqrt(64.0))  # sqrt(8): scale split over q and k
    in_maps = []
    mask = np.zeros((P, 4, CH), np.float32)
    x = np.arange(P)[:, None]
    yy = np.arange(CH)[None, :]
    mask[:, 0, :] = np.where(x - yy >= 0, 0.0, NEG)
    for c in range(8):
        b, g = c // 2, c % 2
        heads = slice(g * HLOC, (g + 1) * HLOC)
        # [H,D,DK] -> [D, HLOC*DK]
        wq_p = (Wq[heads] * rt8).transpose(1, 0, 2).reshape(D, HD)
        wk_p = (Wk[heads] * rt8).transpose(1, 0, 2).reshape(D, HD)
        wv_p = Wv[heads].transpose(1, 0, 2).reshape(D, HD)
        wo_p = Wo[:, g * HD : (g + 1) * HD].T  # [HD, D]
        in_maps.append({
            "xtq": np.ascontiguousarray(Q[b].T).astype(np.float16),
            "xtk": np.ascontiguousarray(K[b].T).astype(np.float16),
            "xtv": np.ascontiguousarray(V[b].T).astype(np.float16),
            "wq": np.ascontiguousarray(wq_p).astype(np.float16),
            "wk": np.ascontiguousarray(wk_p).astype(np.float16),
            "wv": np.ascontiguousarray(wv_p).astype(np.float16),
            "wo": np.ascontiguousarray(wo_p).astype(np.float16),
            "maskadd": mask,
        })
    return in_maps


_NC = []


def kernel(Q, K, V, mask, Wq, Wk, Wv, Wo, bo, _trace=False):
    from concourse.bass_utils import run_bass_kernel_spmd

    Q, K, V = np.asarray(Q), np.asarray(K), np.asarray(V)
    Wq, Wk, Wv = np.asarray(Wq), np.asarray(Wk), np.asarray(Wv)
    Wo, bo = np.asarray(Wo), np.asarray(bo)

    if not _NC:
        _NC.append(build())
    nc = _NC[0]
    in_maps = _prep_inputs(Q, K, V, Wq, Wk, Wv, Wo)
    res = run_bass_kernel_spmd(nc, in_maps, core_ids=list(range(8)), trace=_trace)
    ys = [r["y"] for r in res.results]
    out = np.stack([ys[2 * b] + ys[2 * b + 1] for b in range(B)])
    out = out + bo[None, None, :].astype(np.float32)
    if _trace:
        kernel._last = res
    return out.astype(np.float32)



# revision 6
# speedup vs baseline: 1.0547x; 1.0107x over previous
"""Multi-head attention (B=4, S=2048, D=1024, H=16, causal) on 8 trn2 cores.

Sharding: data-parallel over batch (4) x tensor-parallel over head groups (2).
Core c handles batch b=c//2, heads g=c%2 (8 heads each). Each core computes
its partial output projection; host sums the two partials per batch and adds
the bias.

Per-core pipeline (all matmul inputs fp16, fp32 accumulation):
  1. qT/kT = W.T @ X.T   [512, 2048] (head-major rows), v = X @ Wv [2048, 512]
  2. per (head, 128-row s-block): scores[s,t] = qT.T kT (K=64 matmul),
     causal mask-add on PSUM (DVE), row-max (DVE), exp(bias=-max) with
     accum_out denominator (ACT), normalize (GPSIMD) -> P fp16
  3. P -> P^T via DMA-transpose XBAR (fp16), PV: out^T[dk, s] += v_tile.T P^T
  4. y_partial[s, :] = concat^T.T @ Wo_part  (K=512), fp32 psum -> DRAM

The tensor-engine MM/LDW ISA slots hold a single semaphore wait, so every
matmul's dependencies are funneled through one engine: DMA-loaded tiles get a
full-range in-place DVE "touch" after load, and the P^T tile gets a GPSIMD
touch after the transposes, so each MM waits on at most one proc.
"""

import math

import numpy as np

B, S, D, H = 4, 2048, 1024, 16
DK = 64
HLOC = 8          # heads per core
HD = HLOC * DK    # 512 local concat dims
P = 128
SBLKS = S // P    # 16
CH = 512          # score/psum chunk width
SCHUNKS = S // CH  # 4
KO = D // P       # 8 contraction tiles for projections
MPAIRS = 4        # head pairs per core (qT/kT stored as [128, MPAIRS, S])
NEG = -30000.0


def build():
    import concourse.bass as bass
    import concourse.mybir as mybir
    import concourse.tile as tile
    from concourse import bacc

    fp16 = mybir.dt.float16
    f32 = mybir.dt.float32

    nc = bacc.Bacc()

    xtq = nc.dram_tensor("xtq", [D, S], fp16, kind="ExternalInput")
    xtk = nc.dram_tensor("xtk", [D, S], fp16, kind="ExternalInput")
    xtv = nc.dram_tensor("xtv", [D, S], fp16, kind="ExternalInput")
    wq = nc.dram_tensor("wq", [D, HD], fp16, kind="ExternalInput")
    wk = nc.dram_tensor("wk", [D, HD], fp16, kind="ExternalInput")
    wv = nc.dram_tensor("wv", [D, HD], fp16, kind="ExternalInput")
    wo = nc.dram_tensor("wo", [HD, D], fp16, kind="ExternalInput")
    maskadd = nc.dram_tensor("maskadd", [P, 4, CH], f32, kind="ExternalInput")
    y = nc.dram_tensor("y", [S, D], fp16, kind="ExternalOutput")

    with tile.TileContext(nc) as tc:
        with (
            tc.tile_pool(name="persist", bufs=1) as persist,
            tc.tile_pool(name="pssc", bufs=6, space="PSUM") as pssc,
            tc.tile_pool(name="psmm", bufs=2, space="PSUM") as psmm,
            tc.tile_pool(name="stats", bufs=24) as stats,
        ):
            # ---- constants / weights ----
            mask_sb = persist.tile([P, 4, CH], f32, tag="mask")
            nc.sync.dma_start(out=mask_sb, in_=maskadd[:])

            wq_sb = persist.tile([P, KO, HD], fp16, tag="wq")
            wk_sb = persist.tile([P, KO, HD], fp16, tag="wk")
            wv_sb = persist.tile([P, KO, HD], fp16, tag="wv")
            nc.sync.dma_start(out=wq_sb, in_=wq[:].rearrange("(ko p) n -> p ko n", p=P))
            nc.sync.dma_start(out=wk_sb, in_=wk[:].rearrange("(ko p) n -> p ko n", p=P))
            nc.sync.dma_start(out=wv_sb, in_=wv[:].rearrange("(ko p) n -> p ko n", p=P))
            wo_sb = persist.tile([P, MPAIRS, D], fp16, tag="wo")
            nc.sync.dma_start(out=wo_sb, in_=wo[:].rearrange("(m p) n -> p m n", p=P))

            # ---- persistent activations ----
            qt = persist.tile([P, MPAIRS, S], fp16, tag="qt")   # rows = hd % 128
            kt = persist.tile([P, MPAIRS, S], fp16, tag="kt")
            vv = persist.tile([P, SBLKS, HD], fp16, tag="vv")   # [t%128, t//128, hd]
            outt = persist.tile([P, MPAIRS, S], fp16, tag="outt")  # concat^T

            # ---- phase 1: projections ----
            with tc.tile_pool(name="xt", bufs=2) as xtpool:
                for name, src, wsb, dstq in (
                    ("q", xtq, wq_sb, qt),
                    ("k", xtk, wk_sb, kt),
                ):
                    xsb = xtpool.tile([P, KO, S], fp16, tag="xt")
                    src_r = src[:].rearrange("(ko p) s -> p ko s", p=P)
                    for ko in range(KO):
                        nc.sync.dma_start(
                            out=xsb[:, ko, :], in_=src_r[:, ko, :]
                        )
                    for m in range(MPAIRS):
                        for nch in range(SCHUNKS):
                            ps = psmm.tile([P, CH], f32, tag="ps")
                            for ko in range(KO):
                                nc.tensor.matmul(
                                    ps,
                                    lhsT=wsb[:, ko, m * P : (m + 1) * P],
                                    rhs=xsb[:, ko, nch * CH : (nch + 1) * CH],
                                    start=(ko == 0),
                                    stop=(ko == KO - 1),
                                )
                            nc.vector.tensor_copy(
                                out=dstq[:, m, nch * CH : (nch + 1) * CH], in_=ps
                            )
                # v projection: lhsT = X^T tile, rhs = Wv -> v[t, hd]
                xsb = xtpool.tile([P, KO, S], fp16, tag="xt")
                src_r = xtv[:].rearrange("(ko p) s -> p ko s", p=P)
                for ko in range(KO):
                    nc.sync.dma_start(
                        out=xsb[:, ko, :], in_=src_r[:, ko, :]
                    )
                for tm in range(SBLKS):
                    ps = psmm.tile([P, CH], f32, tag="ps")
                    for ko in range(KO):
                        nc.tensor.matmul(
                            ps,
                            lhsT=xsb[:, ko, tm * P : (tm + 1) * P],
                            rhs=wv_sb[:, ko, :],
                            start=(ko == 0),
                            stop=(ko == KO - 1),
                        )
                    nc.vector.tensor_copy(out=vv[:, tm, :], in_=ps)

            # ---- phase 2: attention, two heads of a pair interleaved ----
            ctx2 = tc.tile_pool(name="escr", bufs=3)
            escr = ctx2.__enter__()
            ctx3 = tc.tile_pool(name="pt", bufs=2)
            ptpool = ctx3.__enter__()
            ctx4 = tc.tile_pool(name="outbuf", bufs=2)
            outbuf = ctx4.__enter__()
            for c in range(SCHUNKS):
                for m in range(MPAIRS):
                    nblk = 4 * (c + 1)
                    pts = [
                        ptpool.tile([P, SBLKS, CH], fp16, tag=f"pt{z}",
                                    name=f"pt{z}")
                        for z in (0, 1)
                    ]
                    for r in range(4):
                        i = 4 * c + r  # s-block index
                        wl = P * (r + 1)  # causal width of the diagonal chunk
                        width = c * CH + wl
                        for z in (0, 1):
                            off = z * 64
                            scz = []
                            for cc in range(c + 1):
                                w = CH if cc < c else wl
                                ps = pssc.tile([P, CH], f32, tag="ps")
                                nc.tensor.matmul(
                                    ps[:, :w],
                                    lhsT=qt[off : off + 64, m, i * P : (i + 1) * P],
                                    rhs=kt[off : off + 64, m, cc * CH : cc * CH + w],
                                    start=True,
                                    stop=True,
                                )
                                scz.append(ps)
                            # causal mask on the diagonal 128-col block
                            nc.vector.tensor_tensor(
                                out=scz[c][:, wl - P : wl],
                                in0=scz[c][:, wl - P : wl],
                                in1=mask_sb[:, 0, :P], op=mybir.AluOpType.add,
                            )
                            # negated row max (exp bias)
                            negmx = stats.tile([P, 1], f32, tag="negmx")
                            if c == 0:
                                nc.vector.reduce_max(
                                    negmx, scz[0][:, :wl],
                                    axis=mybir.AxisListType.X, negate=True,
                                )
                            else:
                                mxarr = stats.tile([P, 4], f32, tag="mxarr")
                                for cc in range(c + 1):
                                    w = CH if cc < c else wl
                                    nc.vector.reduce_max(
                                        mxarr[:, cc : cc + 1], scz[cc][:, :w],
                                        axis=mybir.AxisListType.X,
                                    )
                                nc.vector.reduce_max(
                                    negmx, mxarr[:, 0 : c + 1],
                                    axis=mybir.AxisListType.X, negate=True,
                                )
                            # exp + accumulate denominator
                            ebuf = escr.tile(
                                [P, SCHUNKS * CH], fp16, tag=f"ebuf{z}",
                                name=f"ebuf{z}",
                            )
                            acc = stats.tile([P, 4], f32, tag="acc")
                            for cc in range(c + 1):
                                w = CH if cc < c else wl
                                nc.scalar.activation(
                                    out=ebuf[:, cc * CH : cc * CH + w],
                                    in_=scz[cc][:, :w],
                                    func=mybir.ActivationFunctionType.Exp,
                                    bias=negmx,
                                    scale=1.0,
                                    accum_out=acc[:, cc : cc + 1],
                                )
                            den = stats.tile([P, 1], f32, tag="den")
                            if c == 0:
                                nc.vector.reciprocal(den, acc[:, 0:1])
                            else:
                                nc.vector.reduce_sum(
                                    den, acc[:, 0 : c + 1],
                                    axis=mybir.AxisListType.X,
                                )
                                nc.vector.reciprocal(den, den)
                            # normalize P = E/den; split across DVE / gpsimd
                            nc.gpsimd.tensor_scalar_mul(
                                ebuf[:, 0:width], ebuf[:, 0:width], den
                            )
                            # transpose P[s-block, t] -> P^T[t, s-block cols]
                            nc.sync.dma_start(
                                out=pts[z][:, 0 : i + 1, r * P : (r + 1) * P],
                                in_=ebuf[:, 0:width],
                                transpose=True,
                            )

                    # PV for this s-chunk: out^T[dk, s] = sum_j v_j.T @ P^T_j
                    for z in (0, 1):
                        off = z * 64
                        # zero the two causally-dead corners PV reads
                        nc.gpsimd.memset(pts[z][:, 4 * c + 1, 0:P], 0.0)
                        nc.gpsimd.memset(pts[z][:, 4 * c + 3, 2 * P : 3 * P], 0.0)
                        po = psmm.tile([64, CH], f32, tag="ps")
                        for half in (0, 1):
                            hs = half * 256
                            jmax = 4 * c + 1 if half == 0 else 4 * c + 3
                            for j in range(jmax + 1):
                                nc.tensor.matmul(
                                    po[:, hs : hs + 256],
                                    lhsT=vv[:, j, off + m * P : off + m * P + 64],
                                    rhs=pts[z][:, j, hs : hs + 256],
                                    start=(j == 0),
                                    stop=(j == jmax),
                                )
                        nc.scalar.copy(
                            out=outt[off : off + 64, m, c * CH : (c + 1) * CH],
                            in_=po,
                        )

                # ---- output projection for this chunk's 4 s-blocks ----
                for i in range(4 * c, 4 * c + 4):
                    ysb = outbuf.tile([P, D], fp16, tag="ysb", name="ysb")
                    for nch in range(2):
                        ps = psmm.tile([P, CH], f32, tag="ps", name="ps")
                        for m in range(MPAIRS):
                            nc.tensor.matmul(
                                ps,
                                lhsT=outt[:, m, i * P : (i + 1) * P],
                                rhs=wo_sb[:, m, nch * CH : (nch + 1) * CH],
                                start=(m == 0),
                                stop=(m == MPAIRS - 1),
                            )
                        nc.scalar.copy(
                            out=ysb[:, nch * CH : (nch + 1) * CH], in_=ps
                        )
                    nc.sync.dma_start(out=y[:][i * P : (i + 1) * P, :], in_=ysb)
            ctx4.__exit__(None, None, None)
            ctx3.__exit__(None, None, None)
            ctx2.__exit__(None, None, None)

    nc.finalize()
    return nc


def _prep_inputs(Q, K, V, Wq, Wk, Wv, Wo):
    """Host-side shard + layout prep. Returns list of 8 in_maps."""
    rt8 = math.sqrt(math.sqrt(64.0))  # sqrt(8): scale split over q and k
    in_maps = []
    mask = np.zeros((P, 4, CH), np.float32)
    x = np.arange(P)[:, None]
    yy = np.arange(CH)[None, :]
    mask[:, 0, :] = np.where(x - yy >= 0, 0.0, NEG)
    for c in range(8):
        b, g = c // 2, c % 2
        heads = slice(g * HLOC, (g + 1) * HLOC)
        # [H,D,DK] -> [D, HLOC*DK]
        wq_p = (Wq[heads] * rt8).transpose(1, 0, 2).reshape(D, HD)
        wk_p = (Wk[heads] * rt8).transpose(1, 0, 2).reshape(D, HD)
        wv_p = Wv[heads].transpose(1, 0, 2).reshape(D, HD)
        wo_p = Wo[:, g * HD : (g + 1) * HD].T  # [HD, D]
        in_maps.append({
            "xtq": np.ascontiguousarray(Q[b].T).astype(np.float16),
            "xtk": np.ascontiguousarray(K[b].T).astype(np.float16),
            "xtv": np.ascontiguousarray(V[b].T).astype(np.float16),
            "wq": np.ascontiguousarray(wq_p).astype(np.float16),
            "wk": np.ascontiguousarray(wk_p).astype(np.float16),
            "wv": np.ascontiguousarray(wv_p).astype(np.float16),
            "wo": np.ascontiguousarray(wo_p).astype(np.float16),
            "maskadd": mask,
        })
    return in_maps


_NC = []


def kernel(Q, K, V, mask, Wq, Wk, Wv, Wo, bo, _trace=False):
    from concourse.bass_utils import run_bass_kernel_spmd

    Q, K, V = np.asarray(Q), np.asarray(K), np.asarray(V)
    Wq, Wk, Wv = np.asarray(Wq), np.asarray(Wk), np.asarray(Wv)
    Wo, bo = np.asarray(Wo), np.asarray(bo)

    if not _NC:
        _NC.append(build())
    nc = _NC[0]
    in_maps = _prep_inputs(Q, K, V, Wq, Wk, Wv, Wo)
    res = run_bass_kernel_spmd(nc, in_maps, core_ids=list(range(8)), trace=_trace)
    ys = [r["y"].astype(np.float32) for r in res.results]
    out = np.stack([ys[2 * b] + ys[2 * b + 1] for b in range(B)])
    out = out + bo[None, None, :].astype(np.float32)
    if _trace:
        kernel._last = res
    return out.astype(np.float32)



# revision 7
# speedup vs baseline: 1.0810x; 1.0249x over previous
"""Multi-head attention (B=4, S=2048, D=1024, H=16, causal) on 8 trn2 cores.

Sharding: data-parallel over batch (4) x tensor-parallel over head groups (2).
Core c handles batch b=c//2, heads g=c%2 (8 heads each). Each core computes
its partial output projection; host sums the two partials per batch and adds
the bias.

Per-core pipeline (all matmul inputs fp16, fp32 accumulation):
  1. qT/kT = W.T @ X.T   [512, 2048] (head-major rows), v = X @ Wv [2048, 512]
  2. per (head, 128-row s-block): scores[s,t] = qT.T kT (K=64 matmul),
     causal mask-add on PSUM (DVE), row-max (DVE), exp(bias=-max) with
     accum_out denominator (ACT), normalize (GPSIMD) -> P fp16
  3. P -> P^T via DMA-transpose XBAR (fp16), PV: out^T[dk, s] += v_tile.T P^T
  4. y_partial[s, :] = concat^T.T @ Wo_part  (K=512), fp32 psum -> DRAM

The tensor-engine MM/LDW ISA slots hold a single semaphore wait, so every
matmul's dependencies are funneled through one engine: DMA-loaded tiles get a
full-range in-place DVE "touch" after load, and the P^T tile gets a GPSIMD
touch after the transposes, so each MM waits on at most one proc.
"""

import math

import numpy as np

B, S, D, H = 4, 2048, 1024, 16
DK = 64
HLOC = 8          # heads per core
HD = HLOC * DK    # 512 local concat dims
P = 128
SBLKS = S // P    # 16
CH = 512          # score/psum chunk width
SCHUNKS = S // CH  # 4
KO = D // P       # 8 contraction tiles for projections
MPAIRS = 4        # head pairs per core (qT/kT stored as [128, MPAIRS, S])
NEG = -30000.0


def build():
    import concourse.bass as bass
    import concourse.mybir as mybir
    import concourse.tile as tile
    from concourse import bacc

    fp16 = mybir.dt.float16
    f32 = mybir.dt.float32

    nc = bacc.Bacc()

    xtq = nc.dram_tensor("xtq", [D, S], fp16, kind="ExternalInput")
    xtk = nc.dram_tensor("xtk", [D, S], fp16, kind="ExternalInput")
    xtv = nc.dram_tensor("xtv", [D, S], fp16, kind="ExternalInput")
    wq = nc.dram_tensor("wq", [D, HD], fp16, kind="ExternalInput")
    wk = nc.dram_tensor("wk", [D, HD], fp16, kind="ExternalInput")
    wv = nc.dram_tensor("wv", [D, HD], fp16, kind="ExternalInput")
    wo = nc.dram_tensor("wo", [HD, D], fp16, kind="ExternalInput")
    maskadd = nc.dram_tensor("maskadd", [P, 4, CH], f32, kind="ExternalInput")
    y = nc.dram_tensor("y", [S, D], fp16, kind="ExternalOutput")

    with tile.TileContext(nc) as tc:
        with (
            tc.tile_pool(name="persist", bufs=1) as persist,
            tc.tile_pool(name="pssc", bufs=6, space="PSUM") as pssc,
            tc.tile_pool(name="psmm", bufs=2, space="PSUM") as psmm,
            tc.tile_pool(name="stats", bufs=24) as stats,
        ):
            # ---- constants / weights ----
            wq_sb = persist.tile([P, KO, HD], fp16, tag="wq")
            wk_sb = persist.tile([P, KO, HD], fp16, tag="wk")
            wv_sb = persist.tile([P, KO, HD], fp16, tag="wv")
            wo_sb = persist.tile([P, MPAIRS, D], fp16, tag="wo")
            mask_sb = persist.tile([P, 4, CH], f32, tag="mask")
            nc.sync.dma_start(out=wq_sb, in_=wq[:].rearrange("(ko p) n -> p ko n", p=P))

            # ---- persistent activations ----
            qt = persist.tile([P, MPAIRS, S], fp16, tag="qt")   # rows = hd % 128
            kt = persist.tile([P, MPAIRS, S], fp16, tag="kt")
            vv = persist.tile([P, SBLKS, HD], fp16, tag="vv")   # [t%128, t//128, hd]
            outt = persist.tile([P, MPAIRS, S], fp16, tag="outt")  # concat^T

            # ---- phase 1: projections ----
            with tc.tile_pool(name="xt", bufs=2) as xtpool:
                for name, src, wsb, dstq in (
                    ("q", xtq, wq_sb, qt),
                    ("k", xtk, wk_sb, kt),
                ):
                    xsb = xtpool.tile([P, KO, S], fp16, tag="xt")
                    src_r = src[:].rearrange("(ko p) s -> p ko s", p=P)
                    for ko in range(KO):
                        nc.sync.dma_start(
                            out=xsb[:, ko, :], in_=src_r[:, ko, :]
                        )
                    if name == "q":
                        nc.sync.dma_start(
                            out=wk_sb, in_=wk[:].rearrange("(ko p) n -> p ko n", p=P))
                        nc.sync.dma_start(
                            out=wv_sb, in_=wv[:].rearrange("(ko p) n -> p ko n", p=P))
                        nc.sync.dma_start(
                            out=wo_sb, in_=wo[:].rearrange("(m p) n -> p m n", p=P))
                        nc.sync.dma_start(out=mask_sb, in_=maskadd[:])
                    for m in range(MPAIRS):
                        for nch in range(SCHUNKS):
                            ps = psmm.tile([P, CH], f32, tag="ps")
                            for ko in range(KO):
                                nc.tensor.matmul(
                                    ps,
                                    lhsT=wsb[:, ko, m * P : (m + 1) * P],
                                    rhs=xsb[:, ko, nch * CH : (nch + 1) * CH],
                                    start=(ko == 0),
                                    stop=(ko == KO - 1),
                                )
                            nc.vector.tensor_copy(
                                out=dstq[:, m, nch * CH : (nch + 1) * CH], in_=ps
                            )
                # v projection: lhsT = X^T tile, rhs = Wv -> v[t, hd]
                xsb = xtpool.tile([P, KO, S], fp16, tag="xt")
                src_r = xtv[:].rearrange("(ko p) s -> p ko s", p=P)
                for ko in range(KO):
                    nc.sync.dma_start(
                        out=xsb[:, ko, :], in_=src_r[:, ko, :]
                    )
                for tm in range(SBLKS):
                    ps = psmm.tile([P, CH], f32, tag="ps")
                    for ko in range(KO):
                        nc.tensor.matmul(
                            ps,
                            lhsT=xsb[:, ko, tm * P : (tm + 1) * P],
                            rhs=wv_sb[:, ko, :],
                            start=(ko == 0),
                            stop=(ko == KO - 1),
                        )
                    nc.vector.tensor_copy(out=vv[:, tm, :], in_=ps)

            # ---- phase 2: attention, two heads of a pair interleaved ----
            ctx2 = tc.tile_pool(name="escr", bufs=3)
            escr = ctx2.__enter__()
            ctx3 = tc.tile_pool(name="pt", bufs=2)
            ptpool = ctx3.__enter__()
            ctx4 = tc.tile_pool(name="outbuf", bufs=2)
            outbuf = ctx4.__enter__()
            for c in range(SCHUNKS):
                for m in range(MPAIRS):
                    nblk = 4 * (c + 1)
                    pts = [
                        ptpool.tile([P, SBLKS, CH], fp16, tag=f"pt{z}",
                                    name=f"pt{z}")
                        for z in (0, 1)
                    ]
                    for r in range(4):
                        i = 4 * c + r  # s-block index
                        wl = P * (r + 1)  # causal width of the diagonal chunk
                        width = c * CH + wl
                        for z in (0, 1):
                            off = z * 64
                            scz = []
                            for cc in range(c + 1):
                                w = CH if cc < c else wl
                                ps = pssc.tile([P, CH], f32, tag="ps")
                                nc.tensor.matmul(
                                    ps[:, :w],
                                    lhsT=qt[off : off + 64, m, i * P : (i + 1) * P],
                                    rhs=kt[off : off + 64, m, cc * CH : cc * CH + w],
                                    start=True,
                                    stop=True,
                                )
                                scz.append(ps)
                            # causal mask on the diagonal 128-col block
                            nc.vector.tensor_tensor(
                                out=scz[c][:, wl - P : wl],
                                in0=scz[c][:, wl - P : wl],
                                in1=mask_sb[:, 0, :P], op=mybir.AluOpType.add,
                            )
                            # negated row max (exp bias)
                            negmx = stats.tile([P, 1], f32, tag="negmx")
                            if c == 0:
                                nc.vector.reduce_max(
                                    negmx, scz[0][:, :wl],
                                    axis=mybir.AxisListType.X, negate=True,
                                )
                            else:
                                mxarr = stats.tile([P, 4], f32, tag="mxarr")
                                for cc in range(c + 1):
                                    w = CH if cc < c else wl
                                    nc.vector.reduce_max(
                                        mxarr[:, cc : cc + 1], scz[cc][:, :w],
                                        axis=mybir.AxisListType.X,
                                    )
                                nc.vector.reduce_max(
                                    negmx, mxarr[:, 0 : c + 1],
                                    axis=mybir.AxisListType.X, negate=True,
                                )
                            # exp + accumulate denominator
                            ebuf = escr.tile(
                                [P, SCHUNKS * CH], fp16, tag=f"ebuf{z}",
                                name=f"ebuf{z}",
                            )
                            acc = stats.tile([P, 4], f32, tag="acc")
                            for cc in range(c + 1):
                                w = CH if cc < c else wl
                                nc.scalar.activation(
                                    out=ebuf[:, cc * CH : cc * CH + w],
                                    in_=scz[cc][:, :w],
                                    func=mybir.ActivationFunctionType.Exp,
                                    bias=negmx,
                                    scale=1.0,
                                    accum_out=acc[:, cc : cc + 1],
                                )
                            den = stats.tile([P, 1], f32, tag="den")
                            if c == 0:
                                nc.vector.reciprocal(den, acc[:, 0:1])
                            else:
                                nc.vector.reduce_sum(
                                    den, acc[:, 0 : c + 1],
                                    axis=mybir.AxisListType.X,
                                )
                                nc.vector.reciprocal(den, den)
                            # normalize P = E/den; split across DVE / gpsimd
                            nc.gpsimd.tensor_scalar_mul(
                                ebuf[:, 0:width], ebuf[:, 0:width], den
                            )
                            # transpose P[s-block, t] -> P^T[t, s-block cols]
                            nc.sync.dma_start(
                                out=pts[z][:, 0 : i + 1, r * P : (r + 1) * P],
                                in_=ebuf[:, 0:width],
                                transpose=True,
                            )

                    # PV for this s-chunk: out^T[dk, s] = sum_j v_j.T @ P^T_j
                    for z in (0, 1):
                        off = z * 64
                        # zero the two causally-dead corners PV reads
                        nc.gpsimd.memset(pts[z][:, 4 * c + 1, 0:P], 0.0)
                        nc.gpsimd.memset(pts[z][:, 4 * c + 3, 2 * P : 3 * P], 0.0)
                        po = psmm.tile([64, CH], f32, tag="ps")
                        for half in (0, 1):
                            hs = half * 256
                            jmax = 4 * c + 1 if half == 0 else 4 * c + 3
                            for j in range(jmax + 1):
                                nc.tensor.matmul(
                                    po[:, hs : hs + 256],
                                    lhsT=vv[:, j, off + m * P : off + m * P + 64],
                                    rhs=pts[z][:, j, hs : hs + 256],
                                    start=(j == 0),
                                    stop=(j == jmax),
                                )
                        nc.scalar.copy(
                            out=outt[off : off + 64, m, c * CH : (c + 1) * CH],
                            in_=po,
                        )

                # ---- output projection for this chunk's 4 s-blocks ----
                for i in range(4 * c, 4 * c + 4):
                    ysb = outbuf.tile([P, D], fp16, tag="ysb", name="ysb")
                    for nch in range(2):
                        ps = psmm.tile([P, CH], f32, tag="ps", name="ps")
                        for m in range(MPAIRS):
                            nc.tensor.matmul(
                                ps,
                                lhsT=outt[:, m, i * P : (i + 1) * P],
                                rhs=wo_sb[:, m, nch * CH : (nch + 1) * CH],
                                start=(m == 0),
                                stop=(m == MPAIRS - 1),
                            )
                        nc.scalar.copy(
                            out=ysb[:, nch * CH : (nch + 1) * CH], in_=ps
                        )
                    nc.sync.dma_start(out=y[:][i * P : (i + 1) * P, :], in_=ysb)
            ctx4.__exit__(None, None, None)
            ctx3.__exit__(None, None, None)
            ctx2.__exit__(None, None, None)

    nc.finalize()
    return nc


def _prep_inputs(Q, K, V, Wq, Wk, Wv, Wo):
    """Host-side shard + layout prep. Returns list of 8 in_maps."""
    rt8 = math.sqrt(math.sqrt(64.0))  # sqrt(8): scale split over q and k
    in_maps = []
    mask = np.zeros((P, 4, CH), np.float32)
    x = np.arange(P)[:, None]
    yy = np.arange(CH)[None, :]
    mask[:, 0, :] = np.where(x - yy >= 0, 0.0, NEG)
    for c in range(8):
        b, g = c // 2, c % 2
        heads = slice(g * HLOC, (g + 1) * HLOC)
        # [H,D,DK] -> [D, HLOC*DK]
        wq_p = (Wq[heads] * rt8).transpose(1, 0, 2).reshape(D, HD)
        wk_p = (Wk[heads] * rt8).transpose(1, 0, 2).reshape(D, HD)
        wv_p = Wv[heads].transpose(1, 0, 2).reshape(D, HD)
        wo_p = Wo[:, g * HD : (g + 1) * HD].T  # [HD, D]
        in_maps.append({
            "xtq": np.ascontiguousarray(Q[b].T).astype(np.float16),
            "xtk": np.ascontiguousarray(K[b].T).astype(np.float16),
            "xtv": np.ascontiguousarray(V[b].T).astype(np.float16),
            "wq": np.ascontiguousarray(wq_p).astype(np.float16),
            "wk": np.ascontiguousarray(wk_p).astype(np.float16),
            "wv": np.ascontiguousarray(wv_p).astype(np.float16),
            "wo": np.ascontiguousarray(wo_p).astype(np.float16),
            "maskadd": mask,
        })
    return in_maps


_NC = []


def kernel(Q, K, V, mask, Wq, Wk, Wv, Wo, bo, _trace=False):
    from concourse.bass_utils import run_bass_kernel_spmd

    Q, K, V = np.asarray(Q), np.asarray(K), np.asarray(V)
    Wq, Wk, Wv = np.asarray(Wq), np.asarray(Wk), np.asarray(Wv)
    Wo, bo = np.asarray(Wo), np.asarray(bo)

    if not _NC:
        _NC.append(build())
    nc = _NC[0]
    in_maps = _prep_inputs(Q, K, V, Wq, Wk, Wv, Wo)
    res = run_bass_kernel_spmd(nc, in_maps, core_ids=list(range(8)), trace=_trace)
    ys = [r["y"].astype(np.float32) for r in res.results]
    out = np.stack([ys[2 * b] + ys[2 * b + 1] for b in range(B)])
    out = out + bo[None, None, :].astype(np.float32)
    if _trace:
        kernel._last = res
    return out.astype(np.float32)



# revision 8
# speedup vs baseline: 1.0839x; 1.0027x over previous
"""Multi-head attention (B=4, S=2048, D=1024, H=16, causal) on 8 trn2 cores.

Sharding: data-parallel over batch (4) x tensor-parallel over head groups (2).
Core c handles batch b=c//2, heads g=c%2 (8 heads each). Each core computes
its partial output projection; host sums the two partials per batch and adds
the bias.

Per-core pipeline (all matmul inputs fp16, fp32 accumulation):
  1. qT/kT = W.T @ X.T   [512, 2048] (head-major rows), v = X @ Wv [2048, 512]
  2. per (head, 128-row s-block): scores[s,t] = qT.T kT (K=64 matmul),
     causal mask-add on PSUM (DVE), row-max (DVE), exp(bias=-max) with
     accum_out denominator (ACT), normalize (GPSIMD) -> P fp16
  3. P -> P^T via DMA-transpose XBAR (fp16), PV: out^T[dk, s] += v_tile.T P^T
  4. y_partial[s, :] = concat^T.T @ Wo_part  (K=512), fp32 psum -> DRAM

The tensor-engine MM/LDW ISA slots hold a single semaphore wait, so every
matmul's dependencies are funneled through one engine: DMA-loaded tiles get a
full-range in-place DVE "touch" after load, and the P^T tile gets a GPSIMD
touch after the transposes, so each MM waits on at most one proc.
"""

import math

import numpy as np

B, S, D, H = 4, 2048, 1024, 16
DK = 64
HLOC = 8          # heads per core
HD = HLOC * DK    # 512 local concat dims
P = 128
SBLKS = S // P    # 16
CH = 512          # score/psum chunk width
SCHUNKS = S // CH  # 4
KO = D // P       # 8 contraction tiles for projections
MPAIRS = 4        # head pairs per core (qT/kT stored as [128, MPAIRS, S])
NEG = -30000.0


def build():
    import concourse.bass as bass
    import concourse.mybir as mybir
    import concourse.tile as tile
    from concourse import bacc

    fp16 = mybir.dt.float16
    f32 = mybir.dt.float32

    nc = bacc.Bacc()

    xtq = nc.dram_tensor("xtq", [D, S], fp16, kind="ExternalInput")
    xtk = nc.dram_tensor("xtk", [D, S], fp16, kind="ExternalInput")
    xtv = nc.dram_tensor("xtv", [D, S], fp16, kind="ExternalInput")
    wq = nc.dram_tensor("wq", [D, HD], fp16, kind="ExternalInput")
    wk = nc.dram_tensor("wk", [D, HD], fp16, kind="ExternalInput")
    wv = nc.dram_tensor("wv", [D, HD], fp16, kind="ExternalInput")
    wo = nc.dram_tensor("wo", [HD, D], fp16, kind="ExternalInput")
    maskadd = nc.dram_tensor("maskadd", [P, 4, CH], f32, kind="ExternalInput")
    y = nc.dram_tensor("y", [S, D], fp16, kind="ExternalOutput")

    with tile.TileContext(nc) as tc:
        with (
            tc.tile_pool(name="persist", bufs=1) as persist,
            tc.tile_pool(name="pssc", bufs=6, space="PSUM") as pssc,
            tc.tile_pool(name="psmm", bufs=2, space="PSUM") as psmm,
            tc.tile_pool(name="stats", bufs=24) as stats,
        ):
            # ---- constants / weights ----
            wq_sb = persist.tile([P, KO, HD], fp16, tag="wq")
            wk_sb = persist.tile([P, KO, HD], fp16, tag="wk")
            wv_sb = persist.tile([P, KO, HD], fp16, tag="wv")
            wo_sb = persist.tile([P, MPAIRS, D], fp16, tag="wo")
            mask_sb = persist.tile([P, 4, CH], f32, tag="mask")
            nc.sync.dma_start(out=wq_sb, in_=wq[:].rearrange("(ko p) n -> p ko n", p=P))

            # ---- persistent activations ----
            qt = persist.tile([P, MPAIRS, S], fp16, tag="qt")   # rows = hd % 128
            kt = persist.tile([P, MPAIRS, S], fp16, tag="kt")
            vv = persist.tile([P, SBLKS, HD], fp16, tag="vv")   # [t%128, t//128, hd]
            outt = persist.tile([P, MPAIRS, S], fp16, tag="outt")  # concat^T

            # ---- phase 1: projections ----
            with tc.tile_pool(name="xt", bufs=2) as xtpool:
                for name, src, wsb, dstq in (
                    ("q", xtq, wq_sb, qt),
                    ("k", xtk, wk_sb, kt),
                ):
                    xsb = xtpool.tile([P, KO, S], fp16, tag="xt")
                    src_r = src[:].rearrange("(ko p) s -> p ko s", p=P)
                    for ko in range(KO):
                        nc.sync.dma_start(
                            out=xsb[:, ko, :], in_=src_r[:, ko, :]
                        )
                    if name == "q":
                        nc.sync.dma_start(
                            out=wk_sb, in_=wk[:].rearrange("(ko p) n -> p ko n", p=P))
                        nc.sync.dma_start(
                            out=wv_sb, in_=wv[:].rearrange("(ko p) n -> p ko n", p=P))
                        nc.sync.dma_start(
                            out=wo_sb, in_=wo[:].rearrange("(m p) n -> p m n", p=P))
                        nc.sync.dma_start(out=mask_sb, in_=maskadd[:])
                    for m in range(MPAIRS):
                        for nch in range(SCHUNKS):
                            ps = psmm.tile([P, CH], f32, tag="ps")
                            for ko in range(KO):
                                nc.tensor.matmul(
                                    ps,
                                    lhsT=wsb[:, ko, m * P : (m + 1) * P],
                                    rhs=xsb[:, ko, nch * CH : (nch + 1) * CH],
                                    start=(ko == 0),
                                    stop=(ko == KO - 1),
                                )
                            nc.vector.tensor_copy(
                                out=dstq[:, m, nch * CH : (nch + 1) * CH], in_=ps
                            )
                # v projection: lhsT = X^T tile, rhs = Wv -> v[t, hd]
                xsb = xtpool.tile([P, KO, S], fp16, tag="xt")
                src_r = xtv[:].rearrange("(ko p) s -> p ko s", p=P)
                for ko in range(KO):
                    nc.sync.dma_start(
                        out=xsb[:, ko, :], in_=src_r[:, ko, :]
                    )
                for tm in range(SBLKS):
                    ps = psmm.tile([P, CH], f32, tag="ps")
                    for ko in range(KO):
                        nc.tensor.matmul(
                            ps,
                            lhsT=xsb[:, ko, tm * P : (tm + 1) * P],
                            rhs=wv_sb[:, ko, :],
                            start=(ko == 0),
                            stop=(ko == KO - 1),
                        )
                    nc.vector.tensor_copy(out=vv[:, tm, :], in_=ps)

            # ---- phase 2: attention, two heads of a pair interleaved ----
            ctx2 = tc.tile_pool(name="escr", bufs=3)
            escr = ctx2.__enter__()
            ctx3 = tc.tile_pool(name="pt", bufs=2)
            ptpool = ctx3.__enter__()
            ctx4 = tc.tile_pool(name="outbuf", bufs=2)
            outbuf = ctx4.__enter__()
            def emit_outproj(i):
                ysb = outbuf.tile([P, D], fp16, tag="ysb", name="ysb")
                for nch in range(2):
                    ps = psmm.tile([P, CH], f32, tag="ps", name="ps")
                    for m_ in range(MPAIRS):
                        nc.tensor.matmul(
                            ps,
                            lhsT=outt[:, m_, i * P : (i + 1) * P],
                            rhs=wo_sb[:, m_, nch * CH : (nch + 1) * CH],
                            start=(m_ == 0),
                            stop=(m_ == MPAIRS - 1),
                        )
                    nc.scalar.copy(
                        out=ysb[:, nch * CH : (nch + 1) * CH], in_=ps
                    )
                nc.sync.dma_start(out=y[:][i * P : (i + 1) * P, :], in_=ysb)

            for c in range(SCHUNKS):
                for m in range(MPAIRS):
                    nblk = 4 * (c + 1)
                    pts = [
                        ptpool.tile([P, SBLKS, CH], fp16, tag=f"pt{z}",
                                    name=f"pt{z}")
                        for z in (0, 1)
                    ]
                    for r in range(4):
                        i = 4 * c + r  # s-block index
                        wl = P * (r + 1)  # causal width of the diagonal chunk
                        width = c * CH + wl
                        for z in (0, 1):
                            off = z * 64
                            scz = []
                            for cc in range(c + 1):
                                w = CH if cc < c else wl
                                ps = pssc.tile([P, CH], f32, tag="ps")
                                nc.tensor.matmul(
                                    ps[:, :w],
                                    lhsT=qt[off : off + 64, m, i * P : (i + 1) * P],
                                    rhs=kt[off : off + 64, m, cc * CH : cc * CH + w],
                                    start=True,
                                    stop=True,
                                )
                                scz.append(ps)
                            # causal mask on the diagonal 128-col block
                            nc.vector.tensor_tensor(
                                out=scz[c][:, wl - P : wl],
                                in0=scz[c][:, wl - P : wl],
                                in1=mask_sb[:, 0, :P], op=mybir.AluOpType.add,
                            )
                            # negated row max (exp bias)
                            negmx = stats.tile([P, 1], f32, tag="negmx")
                            if c == 0:
                                nc.vector.reduce_max(
                                    negmx, scz[0][:, :wl],
                                    axis=mybir.AxisListType.X, negate=True,
                                )
                            else:
                                mxarr = stats.tile([P, 4], f32, tag="mxarr")
                                for cc in range(c + 1):
                                    w = CH if cc < c else wl
                                    nc.vector.reduce_max(
                                        mxarr[:, cc : cc + 1], scz[cc][:, :w],
                                        axis=mybir.AxisListType.X,
                                    )
                                nc.vector.reduce_max(
                                    negmx, mxarr[:, 0 : c + 1],
                                    axis=mybir.AxisListType.X, negate=True,
                                )
                            # exp + accumulate denominator
                            ebuf = escr.tile(
                                [P, SCHUNKS * CH], fp16, tag=f"ebuf{z}",
                                name=f"ebuf{z}",
                            )
                            acc = stats.tile([P, 4], f32, tag="acc")
                            for cc in range(c + 1):
                                w = CH if cc < c else wl
                                nc.scalar.activation(
                                    out=ebuf[:, cc * CH : cc * CH + w],
                                    in_=scz[cc][:, :w],
                                    func=mybir.ActivationFunctionType.Exp,
                                    bias=negmx,
                                    scale=1.0,
                                    accum_out=acc[:, cc : cc + 1],
                                )
                            den = stats.tile([P, 1], f32, tag="den")
                            if c == 0:
                                nc.vector.reciprocal(den, acc[:, 0:1])
                            else:
                                nc.vector.reduce_sum(
                                    den, acc[:, 0 : c + 1],
                                    axis=mybir.AxisListType.X,
                                )
                                nc.vector.reciprocal(den, den)
                            # normalize P = E/den; split across DVE / gpsimd
                            nc.gpsimd.tensor_scalar_mul(
                                ebuf[:, 0:width], ebuf[:, 0:width], den
                            )
                            # transpose P[s-block, t] -> P^T[t, s-block cols]
                            nc.sync.dma_start(
                                out=pts[z][:, 0 : i + 1, r * P : (r + 1) * P],
                                in_=ebuf[:, 0:width],
                                transpose=True,
                            )

                    # PV for this s-chunk: out^T[dk, s] = sum_j v_j.T @ P^T_j
                    for z in (0, 1):
                        off = z * 64
                        # zero the two causally-dead corners PV reads
                        nc.gpsimd.memset(pts[z][:, 4 * c + 1, 0:P], 0.0)
                        nc.gpsimd.memset(pts[z][:, 4 * c + 3, 2 * P : 3 * P], 0.0)
                        po = psmm.tile([64, CH], f32, tag="ps")
                        for half in (0, 1):
                            hs = half * 256
                            jmax = 4 * c + 1 if half == 0 else 4 * c + 3
                            for j in range(jmax + 1):
                                nc.tensor.matmul(
                                    po[:, hs : hs + 256],
                                    lhsT=vv[:, j, off + m * P : off + m * P + 64],
                                    rhs=pts[z][:, j, hs : hs + 256],
                                    start=(j == 0),
                                    stop=(j == jmax),
                                )
                        nc.scalar.copy(
                            out=outt[off : off + 64, m, c * CH : (c + 1) * CH],
                            in_=po,
                        )
                    if c > 0:
                        emit_outproj(4 * (c - 1) + m)


            for r in range(4):
                emit_outproj(12 + r)
            ctx4.__exit__(None, None, None)
            ctx3.__exit__(None, None, None)
            ctx2.__exit__(None, None, None)

    nc.finalize()
    return nc


def _prep_inputs(Q, K, V, Wq, Wk, Wv, Wo):
    """Host-side shard + layout prep. Returns list of 8 in_maps."""
    rt8 = math.sqrt(math.sqrt(64.0))  # sqrt(8): scale split over q and k
    in_maps = []
    mask = np.zeros((P, 4, CH), np.float32)
    x = np.arange(P)[:, None]
    yy = np.arange(CH)[None, :]
    mask[:, 0, :] = np.where(x - yy >= 0, 0.0, NEG)
    for c in range(8):
        b, g = c // 2, c % 2
        heads = slice(g * HLOC, (g + 1) * HLOC)
        # [H,D,DK] -> [D, HLOC*DK]
        wq_p = (Wq[heads] * rt8).transpose(1, 0, 2).reshape(D, HD)
        wk_p = (Wk[heads] * rt8).transpose(1, 0, 2).reshape(D, HD)
        wv_p = Wv[heads].transpose(1, 0, 2).reshape(D, HD)
        wo_p = Wo[:, g * HD : (g + 1) * HD].T  # [HD, D]
        in_maps.append({
            "xtq": np.ascontiguousarray(Q[b].T).astype(np.float16),
            "xtk": np.ascontiguousarray(K[b].T).astype(np.float16),
            "xtv": np.ascontiguousarray(V[b].T).astype(np.float16),
            "wq": np.ascontiguousarray(wq_p).astype(np.float16),
            "wk": np.ascontiguousarray(wk_p).astype(np.float16),
            "wv": np.ascontiguousarray(wv_p).astype(np.float16),
            "wo": np.ascontiguousarray(wo_p).astype(np.float16),
            "maskadd": mask,
        })
    return in_maps


_NC = []


def kernel(Q, K, V, mask, Wq, Wk, Wv, Wo, bo, _trace=False):
    from concourse.bass_utils import run_bass_kernel_spmd

    Q, K, V = np.asarray(Q), np.asarray(K), np.asarray(V)
    Wq, Wk, Wv = np.asarray(Wq), np.asarray(Wk), np.asarray(Wv)
    Wo, bo = np.asarray(Wo), np.asarray(bo)

    if not _NC:
        _NC.append(build())
    nc = _NC[0]
    in_maps = _prep_inputs(Q, K, V, Wq, Wk, Wv, Wo)
    res = run_bass_kernel_spmd(nc, in_maps, core_ids=list(range(8)), trace=_trace)
    ys = [r["y"].astype(np.float32) for r in res.results]
    out = np.stack([ys[2 * b] + ys[2 * b + 1] for b in range(B)])
    out = out + bo[None, None, :].astype(np.float32)
    if _trace:
        kernel._last = res
    return out.astype(np.float32)



# revision 9
# speedup vs baseline: 1.1122x; 1.0261x over previous
"""Multi-head attention (B=4, S=2048, D=1024, H=16, causal) on 8 trn2 cores.

Sharding: data-parallel over batch (4) x tensor-parallel over head groups (2).
Core c handles batch b=c//2, heads g=c%2 (8 heads each). Each core computes
its partial output projection; host sums the two partials per batch and adds
the bias.

Per-core pipeline (all matmul inputs fp16, fp32 accumulation):
  1. qT/kT = W.T @ X.T   [512, 2048] (head-major rows), v = X @ Wv [2048, 512]
  2. per (head, 128-row s-block): scores[s,t] = qT.T kT (K=64 matmul),
     causal mask-add on PSUM (DVE), row-max (DVE), exp(bias=-max) with
     accum_out denominator (ACT), normalize (GPSIMD) -> P fp16
  3. P -> P^T via DMA-transpose XBAR (fp16), PV: out^T[dk, s] += v_tile.T P^T
  4. y_partial[s, :] = concat^T.T @ Wo_part  (K=512), fp32 psum -> DRAM

The tensor-engine MM/LDW ISA slots hold a single semaphore wait, so every
matmul's dependencies are funneled through one engine: DMA-loaded tiles get a
full-range in-place DVE "touch" after load, and the P^T tile gets a GPSIMD
touch after the transposes, so each MM waits on at most one proc.
"""

import math

import numpy as np

B, S, D, H = 4, 2048, 1024, 16
DK = 64
HLOC = 8          # heads per core
HD = HLOC * DK    # 512 local concat dims
P = 128
SBLKS = S // P    # 16
CH = 512          # score/psum chunk width
SCHUNKS = S // CH  # 4
KO = D // P       # 8 contraction tiles for projections
MPAIRS = 4        # head pairs per core (qT/kT stored as [128, MPAIRS, S])
NEG = -30000.0


def build():
    import concourse.bass as bass
    import concourse.mybir as mybir
    import concourse.tile as tile
    from concourse import bacc

    fp16 = mybir.dt.float16
    f32 = mybir.dt.float32

    nc = bacc.Bacc()

    xtq = nc.dram_tensor("xtq", [D, S], fp16, kind="ExternalInput")
    xtk = nc.dram_tensor("xtk", [D, S], fp16, kind="ExternalInput")
    xtv = nc.dram_tensor("xtv", [D, S], fp16, kind="ExternalInput")
    wq = nc.dram_tensor("wq", [D, HD], fp16, kind="ExternalInput")
    wk = nc.dram_tensor("wk", [D, HD], fp16, kind="ExternalInput")
    wv = nc.dram_tensor("wv", [D, HD], fp16, kind="ExternalInput")
    wo = nc.dram_tensor("wo", [HD, D], fp16, kind="ExternalInput")
    maskadd = nc.dram_tensor("maskadd", [P, P], fp16, kind="ExternalInput")
    ident = nc.dram_tensor("ident", [P, P], fp16, kind="ExternalInput")
    y = nc.dram_tensor("y", [S, D], fp16, kind="ExternalOutput")

    with tile.TileContext(nc) as tc:
        with (
            tc.tile_pool(name="persist", bufs=1) as persist,
            tc.tile_pool(name="pssc", bufs=6, space="PSUM") as pssc,
            tc.tile_pool(name="psmm", bufs=2, space="PSUM") as psmm,
            tc.tile_pool(name="stats", bufs=24) as stats,
        ):
            # ---- constants / weights ----
            wq_sb = persist.tile([P, KO, HD], fp16, tag="wq")
            wk_sb = persist.tile([P, KO, HD], fp16, tag="wk")
            wv_sb = persist.tile([P, KO, HD], fp16, tag="wv")
            wo_sb = persist.tile([P, MPAIRS, D], fp16, tag="wo")
            mask_sb = persist.tile([P, P], fp16, tag="mask")
            id_sb = persist.tile([P, P], fp16, tag="ident")
            nc.sync.dma_start(out=wq_sb, in_=wq[:].rearrange("(ko p) n -> p ko n", p=P))

            # ---- persistent activations ----
            qt = persist.tile([P, MPAIRS, S], fp16, tag="qt")   # rows = hd % 128
            kt = persist.tile([P, MPAIRS, S], fp16, tag="kt")
            vv = persist.tile([P, SBLKS, HD], fp16, tag="vv")   # [t%128, t//128, hd]
            outt = persist.tile([P, MPAIRS, S], fp16, tag="outt")  # concat^T

            # ---- phase 1: projections ----
            with tc.tile_pool(name="xt", bufs=2) as xtpool:
                for name, src, wsb, dstq in (
                    ("q", xtq, wq_sb, qt),
                    ("k", xtk, wk_sb, kt),
                ):
                    xsb = xtpool.tile([P, KO, S], fp16, tag="xt")
                    src_r = src[:].rearrange("(ko p) s -> p ko s", p=P)
                    for ko in range(KO):
                        nc.sync.dma_start(
                            out=xsb[:, ko, :], in_=src_r[:, ko, :]
                        )
                    if name == "q":
                        nc.sync.dma_start(
                            out=wk_sb, in_=wk[:].rearrange("(ko p) n -> p ko n", p=P))
                        nc.sync.dma_start(
                            out=wv_sb, in_=wv[:].rearrange("(ko p) n -> p ko n", p=P))
                        nc.sync.dma_start(
                            out=wo_sb, in_=wo[:].rearrange("(m p) n -> p m n", p=P))
                        nc.sync.dma_start(out=mask_sb, in_=maskadd[:])
                        nc.sync.dma_start(out=id_sb, in_=ident[:])
                    for m in range(MPAIRS):
                        for nch in range(SCHUNKS):
                            ps = psmm.tile([P, CH], f32, tag="ps")
                            for ko in range(KO):
                                nc.tensor.matmul(
                                    ps,
                                    lhsT=wsb[:, ko, m * P : (m + 1) * P],
                                    rhs=xsb[:, ko, nch * CH : (nch + 1) * CH],
                                    start=(ko == 0),
                                    stop=(ko == KO - 1),
                                )
                            nc.vector.tensor_copy(
                                out=dstq[:, m, nch * CH : (nch + 1) * CH], in_=ps
                            )
                # v projection: lhsT = X^T tile, rhs = Wv -> v[t, hd]
                xsb = xtpool.tile([P, KO, S], fp16, tag="xt")
                src_r = xtv[:].rearrange("(ko p) s -> p ko s", p=P)
                for ko in range(KO):
                    nc.sync.dma_start(
                        out=xsb[:, ko, :], in_=src_r[:, ko, :]
                    )
                for tm in range(SBLKS):
                    ps = psmm.tile([P, CH], f32, tag="ps")
                    for ko in range(KO):
                        nc.tensor.matmul(
                            ps,
                            lhsT=xsb[:, ko, tm * P : (tm + 1) * P],
                            rhs=wv_sb[:, ko, :],
                            start=(ko == 0),
                            stop=(ko == KO - 1),
                        )
                    nc.vector.tensor_copy(out=vv[:, tm, :], in_=ps)

            # ---- phase 2: attention, two heads of a pair interleaved ----
            ctx2 = tc.tile_pool(name="escr", bufs=3)
            escr = ctx2.__enter__()
            ctx3 = tc.tile_pool(name="pt", bufs=2)
            ptpool = ctx3.__enter__()
            ctx4 = tc.tile_pool(name="outbuf", bufs=2)
            outbuf = ctx4.__enter__()
            def emit_outproj(i):
                ysb = outbuf.tile([P, D], fp16, tag="ysb", name="ysb")
                for nch in range(2):
                    ps = psmm.tile([P, CH], f32, tag="ps", name="ps")
                    for m_ in range(MPAIRS):
                        nc.tensor.matmul(
                            ps,
                            lhsT=outt[:, m_, i * P : (i + 1) * P],
                            rhs=wo_sb[:, m_, nch * CH : (nch + 1) * CH],
                            start=(m_ == 0),
                            stop=(m_ == MPAIRS - 1),
                        )
                    nc.scalar.copy(
                        out=ysb[:, nch * CH : (nch + 1) * CH], in_=ps
                    )
                nc.sync.dma_start(out=y[:][i * P : (i + 1) * P, :], in_=ysb)

            for c in range(SCHUNKS):
                for m in range(MPAIRS):
                    nblk = 4 * (c + 1)
                    pts = [
                        ptpool.tile([P, SBLKS, CH], fp16, tag=f"pt{z}",
                                    name=f"pt{z}")
                        for z in (0, 1)
                    ]
                    for r in range(4):
                        i = 4 * c + r  # s-block index
                        wl = P * (r + 1)  # causal width of the diagonal chunk
                        width = c * CH + wl
                        for z in (0, 1):
                            off = z * 64
                            scz = []
                            for cc in range(c + 1):
                                w = CH if cc < c else wl
                                last = cc == c
                                ps = pssc.tile([P, CH], f32, tag="ps")
                                nc.tensor.matmul(
                                    ps[:, :w],
                                    lhsT=qt[off : off + 64, m, i * P : (i + 1) * P],
                                    rhs=kt[off : off + 64, m, cc * CH : cc * CH + w],
                                    start=True,
                                    stop=not last,
                                    skip_group_check=True,
                                )
                                if last:
                                    # causal mask add on PE: I.T @ mask
                                    nc.tensor.matmul(
                                        ps[:, wl - P : wl],
                                        lhsT=id_sb,
                                        rhs=mask_sb,
                                        start=False,
                                        stop=True,
                                        skip_group_check=True,
                                    )
                                scz.append(ps)
                            # negated row max (exp bias)
                            negmx = stats.tile([P, 1], f32, tag="negmx")
                            if c == 0:
                                nc.vector.reduce_max(
                                    negmx, scz[0][:, :wl],
                                    axis=mybir.AxisListType.X, negate=True,
                                )
                            else:
                                mxarr = stats.tile([P, 4], f32, tag="mxarr")
                                for cc in range(c + 1):
                                    w = CH if cc < c else wl
                                    nc.vector.reduce_max(
                                        mxarr[:, cc : cc + 1], scz[cc][:, :w],
                                        axis=mybir.AxisListType.X,
                                    )
                                nc.vector.reduce_max(
                                    negmx, mxarr[:, 0 : c + 1],
                                    axis=mybir.AxisListType.X, negate=True,
                                )
                            # exp + accumulate denominator
                            ebuf = escr.tile(
                                [P, SCHUNKS * CH], fp16, tag=f"ebuf{z}",
                                name=f"ebuf{z}",
                            )
                            acc = stats.tile([P, 4], f32, tag="acc")
                            for cc in range(c + 1):
                                w = CH if cc < c else wl
                                nc.scalar.activation(
                                    out=ebuf[:, cc * CH : cc * CH + w],
                                    in_=scz[cc][:, :w],
                                    func=mybir.ActivationFunctionType.Exp,
                                    bias=negmx,
                                    scale=1.0,
                                    accum_out=acc[:, cc : cc + 1],
                                )
                            den = stats.tile([P, 1], f32, tag="den")
                            if c == 0:
                                nc.vector.reciprocal(den, acc[:, 0:1])
                            else:
                                nc.vector.reduce_sum(
                                    den, acc[:, 0 : c + 1],
                                    axis=mybir.AxisListType.X,
                                )
                                nc.vector.reciprocal(den, den)
                            # normalize P = E/den; split across DVE / gpsimd
                            nc.gpsimd.tensor_scalar_mul(
                                ebuf[:, 0:width], ebuf[:, 0:width], den
                            )
                            # transpose P[s-block, t] -> P^T[t, s-block cols]
                            nc.sync.dma_start(
                                out=pts[z][:, 0 : i + 1, r * P : (r + 1) * P],
                                in_=ebuf[:, 0:width],
                                transpose=True,
                            )

                    # PV for this s-chunk: out^T[dk, s] = sum_j v_j.T @ P^T_j
                    for z in (0, 1):
                        off = z * 64
                        # zero the two causally-dead corners PV reads
                        nc.gpsimd.memset(pts[z][:, 4 * c + 1, 0:P], 0.0)
                        nc.gpsimd.memset(pts[z][:, 4 * c + 3, 2 * P : 3 * P], 0.0)
                        po = psmm.tile([64, CH], f32, tag="ps")
                        for half in (0, 1):
                            hs = half * 256
                            jmax = 4 * c + 1 if half == 0 else 4 * c + 3
                            for j in range(jmax + 1):
                                nc.tensor.matmul(
                                    po[:, hs : hs + 256],
                                    lhsT=vv[:, j, off + m * P : off + m * P + 64],
                                    rhs=pts[z][:, j, hs : hs + 256],
                                    start=(j == 0),
                                    stop=(j == jmax),
                                )
                        nc.scalar.copy(
                            out=outt[off : off + 64, m, c * CH : (c + 1) * CH],
                            in_=po,
                        )
                    if c > 0:
                        emit_outproj(4 * (c - 1) + m)


            for r in range(4):
                emit_outproj(12 + r)
            ctx4.__exit__(None, None, None)
            ctx3.__exit__(None, None, None)
            ctx2.__exit__(None, None, None)

    nc.finalize()
    return nc


def _prep_inputs(Q, K, V, Wq, Wk, Wv, Wo):
    """Host-side shard + layout prep. Returns list of 8 in_maps."""
    rt8 = math.sqrt(math.sqrt(64.0))  # sqrt(8): scale split over q and k
    in_maps = []
    mask = np.where(
        np.arange(P)[:, None] >= np.arange(P)[None, :], 0.0, NEG
    ).astype(np.float16)
    identm = np.eye(P, dtype=np.float16)
    for c in range(8):
        b, g = c // 2, c % 2
        heads = slice(g * HLOC, (g + 1) * HLOC)
        # [H,D,DK] -> [D, HLOC*DK]
        wq_p = (Wq[heads] * rt8).transpose(1, 0, 2).reshape(D, HD)
        wk_p = (Wk[heads] * rt8).transpose(1, 0, 2).reshape(D, HD)
        wv_p = Wv[heads].transpose(1, 0, 2).reshape(D, HD)
        wo_p = Wo[:, g * HD : (g + 1) * HD].T  # [HD, D]
        in_maps.append({
            "xtq": np.ascontiguousarray(Q[b].T).astype(np.float16),
            "xtk": np.ascontiguousarray(K[b].T).astype(np.float16),
            "xtv": np.ascontiguousarray(V[b].T).astype(np.float16),
            "wq": np.ascontiguousarray(wq_p).astype(np.float16),
            "wk": np.ascontiguousarray(wk_p).astype(np.float16),
            "wv": np.ascontiguousarray(wv_p).astype(np.float16),
            "wo": np.ascontiguousarray(wo_p).astype(np.float16),
            "maskadd": mask,
            "ident": identm,
        })
    return in_maps


_NC = []


def kernel(Q, K, V, mask, Wq, Wk, Wv, Wo, bo, _trace=False):
    from concourse.bass_utils import run_bass_kernel_spmd

    Q, K, V = np.asarray(Q), np.asarray(K), np.asarray(V)
    Wq, Wk, Wv = np.asarray(Wq), np.asarray(Wk), np.asarray(Wv)
    Wo, bo = np.asarray(Wo), np.asarray(bo)

    if not _NC:
        _NC.append(build())
    nc = _NC[0]
    in_maps = _prep_inputs(Q, K, V, Wq, Wk, Wv, Wo)
    res = run_bass_kernel_spmd(nc, in_maps, core_ids=list(range(8)), trace=_trace)
    ys = [r["y"].astype(np.float32) for r in res.results]
    out = np.stack([ys[2 * b] + ys[2 * b + 1] for b in range(B)])
    out = out + bo[None, None, :].astype(np.float32)
    if _trace:
        kernel._last = res
    return out.astype(np.float32)



# revision 10
# speedup vs baseline: 1.1494x; 1.0334x over previous
"""Multi-head attention (B=4, S=2048, D=1024, H=16, causal) on 8 trn2 cores.

Sharding: data-parallel over batch (4) x tensor-parallel over head groups (2).
Core c handles batch b=c//2, heads g=c%2 (8 heads each). Each core computes
its partial output projection; host sums the two partials per batch and adds
the bias.

Per-core pipeline (all matmul inputs fp16, fp32 accumulation):
  1. qT/kT = W.T @ X.T   [512, 2048] (head-major rows), v = X @ Wv [2048, 512]
  2. per (head, 128-row s-block): scores[s,t] = qT.T kT (K=64 matmul),
     causal mask-add on PSUM (DVE), row-max (DVE), exp(bias=-max) with
     accum_out denominator (ACT), normalize (GPSIMD) -> P fp16
  3. P -> P^T via DMA-transpose XBAR (fp16), PV: out^T[dk, s] += v_tile.T P^T
  4. y_partial[s, :] = concat^T.T @ Wo_part  (K=512), fp32 psum -> DRAM

The tensor-engine MM/LDW ISA slots hold a single semaphore wait, so every
matmul's dependencies are funneled through one engine: DMA-loaded tiles get a
full-range in-place DVE "touch" after load, and the P^T tile gets a GPSIMD
touch after the transposes, so each MM waits on at most one proc.
"""

import math

import numpy as np

B, S, D, H = 4, 2048, 1024, 16
DK = 64
HLOC = 8          # heads per core
HD = HLOC * DK    # 512 local concat dims
P = 128
SBLKS = S // P    # 16
CH = 512          # score/psum chunk width
SCHUNKS = S // CH  # 4
KO = D // P       # 8 contraction tiles for projections
MPAIRS = 4        # head pairs per core (qT/kT stored as [128, MPAIRS, S])
NEG = -30000.0


def build():
    import concourse.bass as bass
    import concourse.mybir as mybir
    import concourse.tile as tile
    from concourse import bacc

    fp16 = mybir.dt.float16
    f32 = mybir.dt.float32

    nc = bacc.Bacc()

    xtq = nc.dram_tensor("xtq", [D, S], fp16, kind="ExternalInput")
    xtk = nc.dram_tensor("xtk", [D, S], fp16, kind="ExternalInput")
    xtv = nc.dram_tensor("xtv", [D, S], fp16, kind="ExternalInput")
    wq = nc.dram_tensor("wq", [D, HD], fp16, kind="ExternalInput")
    wk = nc.dram_tensor("wk", [D, HD], fp16, kind="ExternalInput")
    wv = nc.dram_tensor("wv", [D, HD], fp16, kind="ExternalInput")
    wo = nc.dram_tensor("wo", [HD, D], fp16, kind="ExternalInput")
    maskadd = nc.dram_tensor("maskadd", [P, P], fp16, kind="ExternalInput")
    ident = nc.dram_tensor("ident", [P, P], fp16, kind="ExternalInput")
    y = nc.dram_tensor("y", [S, D], fp16, kind="ExternalOutput")

    with tile.TileContext(nc) as tc:
        with (
            tc.tile_pool(name="persist", bufs=1) as persist,
            tc.tile_pool(name="pssc", bufs=6, space="PSUM") as pssc,
            tc.tile_pool(name="psmm", bufs=2, space="PSUM") as psmm,
            tc.tile_pool(name="stats", bufs=24) as stats,
        ):
            # ---- constants / weights ----
            wq_sb = persist.tile([P, KO, HD], fp16, tag="wq")
            wk_sb = persist.tile([P, KO, HD], fp16, tag="wk")
            wv_sb = persist.tile([P, KO, HD], fp16, tag="wv")
            wo_sb = persist.tile([P, MPAIRS, D], fp16, tag="wo")
            mask_sb = persist.tile([P, P], fp16, tag="mask")
            id_sb = persist.tile([P, P], fp16, tag="ident")
            nc.sync.dma_start(out=wq_sb, in_=wq[:].rearrange("(ko p) n -> p ko n", p=P))

            # ---- persistent activations ----
            qt = persist.tile([P, MPAIRS, S], fp16, tag="qt")   # rows = hd % 128
            kt = persist.tile([P, MPAIRS, S], fp16, tag="kt")
            vv = persist.tile([P, SBLKS, HD], fp16, tag="vv")   # [t%128, t//128, hd]
            outt = persist.tile([P, MPAIRS, S], fp16, tag="outt")  # concat^T

            # ---- phase 1: projections ----
            with tc.tile_pool(name="xt", bufs=2) as xtpool:
                for name, src, wsb, dstq in (
                    ("q", xtq, wq_sb, qt),
                    ("k", xtk, wk_sb, kt),
                ):
                    xsb = xtpool.tile([P, KO, S], fp16, tag="xt")
                    src_r = src[:].rearrange("(ko p) s -> p ko s", p=P)
                    for ko in range(KO):
                        nc.sync.dma_start(
                            out=xsb[:, ko, :], in_=src_r[:, ko, :]
                        )
                    if name == "q":
                        nc.sync.dma_start(
                            out=wk_sb, in_=wk[:].rearrange("(ko p) n -> p ko n", p=P))
                        nc.sync.dma_start(
                            out=wv_sb, in_=wv[:].rearrange("(ko p) n -> p ko n", p=P))
                        nc.sync.dma_start(
                            out=wo_sb, in_=wo[:].rearrange("(m p) n -> p m n", p=P))
                        nc.sync.dma_start(out=mask_sb, in_=maskadd[:])
                        nc.sync.dma_start(out=id_sb, in_=ident[:])
                    for m in range(MPAIRS):
                        for nch in range(SCHUNKS):
                            ps = psmm.tile([P, CH], f32, tag="ps")
                            for ko in range(KO):
                                nc.tensor.matmul(
                                    ps,
                                    lhsT=wsb[:, ko, m * P : (m + 1) * P],
                                    rhs=xsb[:, ko, nch * CH : (nch + 1) * CH],
                                    start=(ko == 0),
                                    stop=(ko == KO - 1),
                                )
                            nc.vector.tensor_copy(
                                out=dstq[:, m, nch * CH : (nch + 1) * CH], in_=ps
                            )
                # v projection: lhsT = X^T tile, rhs = Wv -> v[t, hd]
                xsb = xtpool.tile([P, KO, S], fp16, tag="xt")
                src_r = xtv[:].rearrange("(ko p) s -> p ko s", p=P)
                for ko in range(KO):
                    nc.sync.dma_start(
                        out=xsb[:, ko, :], in_=src_r[:, ko, :]
                    )
                for tm in range(SBLKS):
                    ps = psmm.tile([P, CH], f32, tag="ps")
                    for ko in range(KO):
                        nc.tensor.matmul(
                            ps,
                            lhsT=xsb[:, ko, tm * P : (tm + 1) * P],
                            rhs=wv_sb[:, ko, :],
                            start=(ko == 0),
                            stop=(ko == KO - 1),
                        )
                    nc.vector.tensor_copy(out=vv[:, tm, :], in_=ps)

            # ---- phase 2: attention, two heads of a pair interleaved ----
            ctx2 = tc.tile_pool(name="escr", bufs=4)
            escr = ctx2.__enter__()
            ctx3 = tc.tile_pool(name="pt", bufs=2)
            ptpool = ctx3.__enter__()
            ctx4 = tc.tile_pool(name="outbuf", bufs=3)
            outbuf = ctx4.__enter__()
            def emit_outproj(i):
                ysb = outbuf.tile([P, D], fp16, tag="ysb", name="ysb")
                for nch in range(2):
                    ps = psmm.tile([P, CH], f32, tag="ps", name="ps")
                    for m_ in range(MPAIRS):
                        nc.tensor.matmul(
                            ps,
                            lhsT=outt[:, m_, i * P : (i + 1) * P],
                            rhs=wo_sb[:, m_, nch * CH : (nch + 1) * CH],
                            start=(m_ == 0),
                            stop=(m_ == MPAIRS - 1),
                        )
                    nc.scalar.copy(
                        out=ysb[:, nch * CH : (nch + 1) * CH], in_=ps
                    )
                nc.sync.dma_start(out=y[:][i * P : (i + 1) * P, :], in_=ysb)

            for c in range(SCHUNKS):
                for m in range(MPAIRS):
                    nblk = 4 * (c + 1)
                    pts = [
                        ptpool.tile([P, SBLKS, CH], fp16, tag=f"pt{z}",
                                    name=f"pt{z}", bufs=2)
                        for z in (0, 1)
                    ]
                    for r in range(4):
                        i = 4 * c + r  # s-block index
                        wl = P * (r + 1)  # causal width of the diagonal chunk
                        width = c * CH + wl
                        for z in (0, 1):
                            off = z * 64
                            scz = []
                            for cc in range(c + 1):
                                w = CH if cc < c else wl
                                last = cc == c
                                ps = pssc.tile([P, CH], f32, tag="ps")
                                nc.tensor.matmul(
                                    ps[:, :w],
                                    lhsT=qt[off : off + 64, m, i * P : (i + 1) * P],
                                    rhs=kt[off : off + 64, m, cc * CH : cc * CH + w],
                                    start=True,
                                    stop=not last,
                                    skip_group_check=True,
                                )
                                if last:
                                    # causal mask add on PE: I.T @ mask
                                    nc.tensor.matmul(
                                        ps[:, wl - P : wl],
                                        lhsT=id_sb,
                                        rhs=mask_sb,
                                        start=False,
                                        stop=True,
                                        skip_group_check=True,
                                    )
                                scz.append(ps)
                            # negated row max (exp bias)
                            negmx = stats.tile([P, 1], f32, tag="negmx")
                            if c == 0:
                                nc.vector.reduce_max(
                                    negmx, scz[0][:, :wl],
                                    axis=mybir.AxisListType.X, negate=True,
                                )
                            else:
                                mxarr = stats.tile([P, 4], f32, tag="mxarr")
                                for cc in range(c + 1):
                                    w = CH if cc < c else wl
                                    nc.vector.reduce_max(
                                        mxarr[:, cc : cc + 1], scz[cc][:, :w],
                                        axis=mybir.AxisListType.X,
                                    )
                                nc.vector.reduce_max(
                                    negmx, mxarr[:, 0 : c + 1],
                                    axis=mybir.AxisListType.X, negate=True,
                                )
                            # exp + accumulate denominator
                            ebuf = escr.tile(
                                [P, SCHUNKS * CH], fp16, tag=f"ebuf{z}",
                                name=f"ebuf{z}",
                            )
                            acc = stats.tile([P, 4], f32, tag="acc")
                            for cc in range(c + 1):
                                w = CH if cc < c else wl
                                nc.scalar.activation(
                                    out=ebuf[:, cc * CH : cc * CH + w],
                                    in_=scz[cc][:, :w],
                                    func=mybir.ActivationFunctionType.Exp,
                                    bias=negmx,
                                    scale=1.0,
                                    accum_out=acc[:, cc : cc + 1],
                                )
                            den = stats.tile([P, 1], f32, tag="den")
                            if c == 0:
                                nc.vector.reciprocal(den, acc[:, 0:1])
                            else:
                                nc.vector.reduce_sum(
                                    den, acc[:, 0 : c + 1],
                                    axis=mybir.AxisListType.X,
                                )
                                nc.vector.reciprocal(den, den)
                            # normalize P = E/den; split across DVE / gpsimd
                            nc.gpsimd.tensor_scalar_mul(
                                ebuf[:, 0:width], ebuf[:, 0:width], den
                            )
                            # transpose P[s-block, t] -> P^T[t, s-block cols]
                            nc.sync.dma_start(
                                out=pts[z][:, 0 : i + 1, r * P : (r + 1) * P],
                                in_=ebuf[:, 0:width],
                                transpose=True,
                            )

                    # PV for this s-chunk: out^T[dk, s] = sum_j v_j.T @ P^T_j
                    for z in (0, 1):
                        off = z * 64
                        # zero the two causally-dead corners PV reads
                        nc.gpsimd.memset(pts[z][:, 4 * c + 1, 0:P], 0.0)
                        nc.gpsimd.memset(pts[z][:, 4 * c + 3, 2 * P : 3 * P], 0.0)
                        po = psmm.tile([64, CH], f32, tag="ps")
                        for half in (0, 1):
                            hs = half * 256
                            jmax = 4 * c + 1 if half == 0 else 4 * c + 3
                            for j in range(jmax + 1):
                                nc.tensor.matmul(
                                    po[:, hs : hs + 256],
                                    lhsT=vv[:, j, off + m * P : off + m * P + 64],
                                    rhs=pts[z][:, j, hs : hs + 256],
                                    start=(j == 0),
                                    stop=(j == jmax),
                                )
                        nc.scalar.copy(
                            out=outt[off : off + 64, m, c * CH : (c + 1) * CH],
                            in_=po,
                        )
                    if c > 0:
                        emit_outproj(4 * (c - 1) + m)


            for r in range(4):
                emit_outproj(12 + r)
            ctx4.__exit__(None, None, None)
            ctx3.__exit__(None, None, None)
            ctx2.__exit__(None, None, None)

    nc.finalize()
    return nc


def _prep_inputs(Q, K, V, Wq, Wk, Wv, Wo):
    """Host-side shard + layout prep. Returns list of 8 in_maps."""
    rt8 = math.sqrt(math.sqrt(64.0))  # sqrt(8): scale split over q and k
    in_maps = []
    mask = np.where(
        np.arange(P)[:, None] >= np.arange(P)[None, :], 0.0, NEG
    ).astype(np.float16)
    identm = np.eye(P, dtype=np.float16)
    for c in range(8):
        b, g = c // 2, c % 2
        heads = slice(g * HLOC, (g + 1) * HLOC)
        # [H,D,DK] -> [D, HLOC*DK]
        wq_p = (Wq[heads] * rt8).transpose(1, 0, 2).reshape(D, HD)
        wk_p = (Wk[heads] * rt8).transpose(1, 0, 2).reshape(D, HD)
        wv_p = Wv[heads].transpose(1, 0, 2).reshape(D, HD)
        wo_p = Wo[:, g * HD : (g + 1) * HD].T  # [HD, D]
        in_maps.append({
            "xtq": np.ascontiguousarray(Q[b].T).astype(np.float16),
            "xtk": np.ascontiguousarray(K[b].T).astype(np.float16),
            "xtv": np.ascontiguousarray(V[b].T).astype(np.float16),
            "wq": np.ascontiguousarray(wq_p).astype(np.float16),
            "wk": np.ascontiguousarray(wk_p).astype(np.float16),
            "wv": np.ascontiguousarray(wv_p).astype(np.float16),
            "wo": np.ascontiguousarray(wo_p).astype(np.float16),
            "maskadd": mask,
            "ident": identm,
        })
    return in_maps


_NC = []


def kernel(Q, K, V, mask, Wq, Wk, Wv, Wo, bo, _trace=False):
    from concourse.bass_utils import run_bass_kernel_spmd

    Q, K, V = np.asarray(Q), np.asarray(K), np.asarray(V)
    Wq, Wk, Wv = np.asarray(Wq), np.asarray(Wk), np.asarray(Wv)
    Wo, bo = np.asarray(Wo), np.asarray(bo)

    if not _NC:
        _NC.append(build())
    nc = _NC[0]
    in_maps = _prep_inputs(Q, K, V, Wq, Wk, Wv, Wo)
    res = run_bass_kernel_spmd(nc, in_maps, core_ids=list(range(8)), trace=_trace)
    ys = [r["y"].astype(np.float32) for r in res.results]
    out = np.stack([ys[2 * b] + ys[2 * b + 1] for b in range(B)])
    out = out + bo[None, None, :].astype(np.float32)
    if _trace:
        kernel._last = res
    return out.astype(np.float32)

